# revision 30
# baseline (speedup 1.0000x reference)
"""Trainium2 Bass kernel for a single-head transformer decoder layer.

Model (per batch element, S=2048, E=1024, F=4096):
    xn  = LN(tgt);  sa = causal_attn(xn)       ; h   = tgt + sa
    xn2 = LN(h);    ca = cross_attn(xn2, src)  ; h  += ca
    xn3 = LN(h);    ff = relu(xn3@W1.T)@W2.T   ; out = h + ff

Sharding: 8 cores = 4 batches x 2-way query-row split.  Core c owns batch
b=c//2 and interleaved 128-row chunks g = 2*j + (c%2), j=0..7 (zig-zag, so
causal-attention work is balanced across the pair).  K/V projections over
all 2048 rows are duplicated within each pair; no collectives.

On-chip layout: activations are stored transposed [feature(part), row(free)],
which lets every matmul in the layer run without any on-chip transpose:
  - proj:    out_T[f,r]   = mm(lhsT=W_T[e,f] blk,  rhs=x_T[e,r])
  - V:       V_nat[r,v]   = mm(lhsT=x_T[e,r] blk,  rhs=W_T[e,v])
  - scores:  s_T[kr,qr]   = mm(lhsT=K_T[e,kr] blk, rhs=Q_T[e,qr])
  - softmax: exp in-place (no max-sub needed; scores are O(1)), column sums
             via ones-matmul (M=128 -> pre-broadcast), causal mask as
             additive bf16 input data
  - AV:      a_T[af,qr]   = mm(lhsT=V_nat[kr,af] blk, rhs=expT[kr,qr])
  - the softmax denominator is divided out after the O-projection, fused
    into the residual add
LN gain and the 1/sqrt(E) score scale are folded into the projection
weights on the host (exact); all biases in this problem are zero.  Q1 is
projected straight from the full-row LN1 output via a strided rhs AP (each
core's owned rows are pre-swizzled to the even 128-col blocks of every
512-chunk), so no separate owned-row LN pass is needed.

Memory: SBUF is managed as a handful of program-long pools whose tags act
as free-slot rings; successive logical tensors (k1->k2->hft, q1->attn1->
xn2->q2->attn2->xn3, the eight 2MB weight images + the w2 stream, the
xn/exp/src-chunk 1MB tiles) reuse slots with WAR deps that naturally time
each prefetch DMA right when its slot's last reader finishes.  This keeps
every large DMA at least one phase ahead of its consumer, which is what
keeps the PE from ever going idle (and from HAM-rethrottling).

Numerics: matmul operands bf16; PSUM accumulation, LN stats and softmax
sums stay fp32.  The residual stream h is bf16 in SBUF (two bf16 roundings
of an O(1) stream, well inside the 2e-2 budget); the final residual add
runs in fp32 and the output is fp32.
"""

import os
import sys

import numpy as np

for _p in ("/opt/trn_rl_repo", "/root/.axon_site/_ro/trn_rl_repo"):
    if os.path.isdir(_p) and _p not in sys.path:
        sys.path.insert(0, _p)

import ml_dtypes  # noqa: E402

import concourse.bass as bass  # noqa: E402
import concourse.tile as tile  # noqa: E402
from concourse import bacc, mybir  # noqa: E402
from concourse.bass_utils import run_bass_kernel_spmd  # noqa: E402

E = 1024
S = 2048
B = 4
F = 4096
P = 128
NE = E // P          # 8 feature blocks
NF = F // P          # 32 ff blocks
NKB = S // P         # 16 key-row blocks
RO = 1024            # owned query rows per core
NCORES = 8

F32 = mybir.dt.float32
BF16 = mybir.dt.bfloat16
BF = ml_dtypes.bfloat16
ALU = mybir.AluOpType
ACT_F = mybir.ActivationFunctionType

NEG = -1e30

_NC_CACHE = {}
LAST_RESULTS = None  # BassKernelResults of the most recent hardware run


def _build_program():
    """Emit the single SPMD program (identical for all 8 cores)."""
    nc = bacc.Bacc(
        "TRN2",
        target_bir_lowering=False,
        debug=False,
        enable_asserts=False,
        num_devices=NCORES,
    )

    d = {}
    d["tgt_t"] = nc.dram_tensor("tgt_t", [4, NE, P, 512], BF16, kind="ExternalInput")
    d["tgto"] = nc.dram_tensor("tgto", [P, NE * RO], BF16, kind="ExternalInput")
    d["src_t"] = nc.dram_tensor("src_t", [4, NE, P, 512], BF16, kind="ExternalInput")
    d["mask"] = nc.dram_tensor("mask", [2, P, 8 * 512], BF16, kind="ExternalInput")
    for w in ("wq1", "wk1", "wv1", "wo1", "wq2", "wk2", "wv2", "wo2"):
        d[w] = nc.dram_tensor(w, [P, NE * E], BF16, kind="ExternalInput")
    d["w1"] = nc.dram_tensor("w1", [NF, P, NE * P], BF16, kind="ExternalInput")
    d["w2"] = nc.dram_tensor("w2", [NE, P, NF * P], BF16, kind="ExternalInput")
    d["out_t"] = nc.dram_tensor("out_t", [P, NE * RO], F32, kind="ExternalOutput")

    with tile.TileContext(nc) as tc:
        with nc.allow_low_precision(
                reason="bf16 LN stats / softmax inv are within the 2e-2 "
                       "relative-error budget (validated in sim)"):
            _emit(tc, {k: v.ap() for k, v in d.items()})

    nc.compile()
    return nc


def _emit(tc, d):
    nc = tc.nc

    # --- PSUM: one pool, 8 banks total across tags -------------------------
    ps = tc.alloc_tile_pool(name="ps", bufs=1, space="PSUM")

    def ps_tile(name, tag, bufs, shape=(P, 512)):
        return ps.tile(list(shape), F32, name=name, tag=tag, bufs=bufs)

    # --- SBUF: program-long pools; tags are free-slot rings ----------------
    def pool(name, bufs=1, side="left"):
        return tc.alloc_tile_pool(name=name, bufs=bufs, side=side)

    consts = pool("consts")
    ones_1 = consts.tile([P, P], BF16, name="ones_1", tag="ones_1")
    nc.vector.memset(ones_1[:], 1.0)
    # 1/E (= 2^-10, exact in bf16) folded into the LN stat sums
    ones_m = consts.tile([P, P], BF16, name="ones_m", tag="ones_m")
    nc.vector.memset(ones_m[:], 1.0 / E)
    eps_t = consts.tile([P, 1], F32, name="eps_t", tag="eps")
    nc.vector.memset(eps_t[:], 1e-5)

    tmp = pool("tmp", bufs=1)        # LN chain scratch, tags t0/t1 (f32)
    sq8 = pool("sq8", bufs=4)        # bf16 squares (DVE) for LN sum(x^2)
    statp = pool("statp", bufs=2)    # LN A/B stat tiles (bf16, 2 rc in flight)
    invp = pool("invp", bufs=1)      # softmax 1/sum tiles (bf16, tags i0/i1)
    st5 = pool("st5", bufs=8)        # [P,512] bf16 stream: tgt-in/mask/res
    outp = pool("outp", bufs=1)      # [P,512] f32 output staging
    warena = pool("warena", bufs=3)  # 2MB slots: 8 proj weights + w2 stream
    kvp = pool("kvp", bufs=2)        # 4MB slots: k1,v1 -> k2,v2 -> hft a/b
    qxp = pool("qxp", bufs=2)        # 2MB slots: q1,attn1 -> xn2,q2 -> attn2,xn3
    xep = pool("xep", bufs=2)        # 1MB slots: xn chunks, exp tiles, src chunks
    hpool = pool("hpool")            # residual stream h (bf16, 2MB)
    w1p = pool("w1p", bufs=3)        # 256KB slots: w1 stream

    h = hpool.tile([P, NE * RO], BF16, name="h", tag="h")

    def w_tile(nm):
        t = warena.tile([P, NE * E], BF16, name=nm, tag="w")
        nc.sync.dma_start(t[:], d[nm][:])
        return t

    def ln_stats(get_x, get_sq, a_tile, b_tile, prefix):
        """Per-row LN stats over one transposed 512-chunk.

        get_x(eb) -> [P,512] bf16 AP; get_sq(eb) -> [P,512] bf16 AP of x^2.
        Fills a_tile = rsqrt(var+eps), b_tile = mean * a  (bf16, broadcast
        along partitions by the 1/E-scaled ones-matmul).
        """
        mu = ps_tile(f"{prefix}sx", "sx", 2)     # mean (1/E in ones_m)
        ex2 = ps_tile(f"{prefix}sxx", "sxx", 2)  # E[x^2]
        for eb in range(NE):
            x = get_x(eb)
            sq = get_sq(eb)
            nc.tensor.matmul(mu[:], ones_m[:], x,
                             start=(eb == 0), stop=(eb == NE - 1))
            nc.tensor.matmul(ex2[:], ones_m[:], sq,
                             start=(eb == 0), stop=(eb == NE - 1))
        v = tmp.tile([P, 512], F32, name=f"{prefix}v", tag="t1")
        nc.scalar.square(v[:], mu[:])
        nc.vector.scalar_tensor_tensor(
            v[:], ex2[:], 1.0, v[:], ALU.mult, ALU.subtract)
        nc.scalar.activation(v[:], v[:], ACT_F.Sqrt, bias=eps_t[:])
        nc.vector.reciprocal_approx_fast(v[:], v[:])
        nc.scalar.copy(a_tile[:], v[:])
        nc.vector.tensor_mul(b_tile[:], mu[:], a_tile[:])

    def ln_apply(dst, src_ap, a, bv, prefix):
        """dst (bf16) = src*A - Bv (bf16 throughout for DVE fast modes)."""
        t = tmp.tile([P, 512], BF16, name=f"{prefix}ap", tag="t0")
        nc.vector.tensor_mul(t[:], src_ap, a[:])
        nc.vector.tensor_sub(dst, t[:], bv[:])

    def stat_tiles(nm):
        a = statp.tile([P, 512], BF16, name=f"{nm}A", tag="A")
        b = statp.tile([P, 512], BF16, name=f"{nm}B", tag="B")
        return a, b

    # ============== phase A: LN1 over all rows -> K1, V1, Q1 ===============
    k1 = kvp.tile([P, NE * S], BF16, name="k1", tag="kv")
    v1 = kvp.tile([P, NKB * E], BF16, name="v1", tag="kv")
    q1 = qxp.tile([P, NE * RO], BF16, name="q1", tag="qx")

    T = [None] * 4

    def load_rc(rc):
        tiles = []
        for eb in range(NE):
            t = st5.tile([P, 512], BF16, name=f"tgt{rc}_{eb}", tag="s5")
            nc.sync.dma_start(t[:], d["tgt_t"][rc, eb])
            tiles.append(t)
        T[rc] = tiles

    # DMA order: first stats chunk, then the phase-A weights
    load_rc(0)
    wk1 = w_tile("wk1")
    load_rc(1)
    wv1 = w_tile("wv1")
    wq1 = w_tile("wq1")

    def stats1(rc):
        sqs = []
        for eb in range(NE):
            sq = sq8.tile([P, 512], BF16, name=f"l1sq{rc}_{eb}", tag="sq")
            nc.gpsimd.tensor_mul(sq[:], T[rc][eb][:], T[rc][eb][:])
            sqs.append(sq)
        a, b = stat_tiles(f"l1f{rc}")
        ln_stats(lambda eb: T[rc][eb][:], lambda eb: sqs[eb][:], a, b,
                 f"l1f{rc}")
        return a, b

    xn = [None] * 4

    def apply1(rc, ab):
        a, b = ab
        x = xep.tile([P, NE * 512], BF16, name=f"xn{rc}", tag="xe")
        for eb in range(NE):
            ln_apply(x[:, eb * 512:eb * 512 + 512], T[rc][eb][:], a, b,
                     f"l1a{rc}")
        xn[rc] = x

    def k1_proj(rc):
        for kf in range(NE):
            kp = ps_tile("kp", "mm", 2)
            for eb in range(NE):
                nc.tensor.matmul(
                    kp[:],
                    wk1[:, eb * E + kf * P:eb * E + kf * P + P],
                    xn[rc][:, eb * 512:eb * 512 + 512],
                    start=(eb == 0), stop=(eb == NE - 1))
            nc.scalar.copy(k1[:, kf * S + rc * 512:kf * S + rc * 512 + 512],
                           kp[:])

    def v1_proj(rc):
        for rb in range(4):
            for vf in range(2):
                vp = ps_tile("vp", "mm", 2)
                for eb in range(NE):
                    nc.tensor.matmul(
                        vp[:],
                        xn[rc][:, eb * 512 + rb * P:eb * 512 + rb * P + P],
                        wv1[:, eb * E + vf * 512:eb * E + vf * 512 + 512],
                        start=(eb == 0), stop=(eb == NE - 1))
                o = (rc * 4 + rb) * E + vf * 512
                nc.scalar.copy(v1[:, o:o + 512], vp[:])

    def q1_proj(c):
        # own chunk c (512 cols) = the even 128-col blocks of xn[2c],
        # xn[2c+1] (the host pre-swizzles tgt so each core's owned rows land
        # at even block positions; mask/tgto follow the swizzle).
        for half in range(2):
            rc = 2 * c + half
            xv = xn[rc][:].rearrange("p (e b t c) -> p e t b c",
                                     e=NE, b=2, t=2, c=P)
            for fblk in range(NE):
                qp = ps_tile("qp", "mm", 2, shape=(P, 256))
                for eb in range(NE):
                    nc.tensor.matmul(
                        qp[:],
                        wq1[:, eb * E + fblk * P:eb * E + fblk * P + P],
                        xv[:, eb, 0],
                        start=(eb == 0), stop=(eb == NE - 1))
                o = fblk * RO + c * 512 + half * 256
                nc.scalar.copy(q1[:, o:o + 256], qp[:])

    apply1(0, stats1(0))
    apply1(1, stats1(1))
    k1_proj(0)
    v1_proj(0)
    q1_proj(0)
    load_rc(2)
    apply1(2, stats1(2))
    k1_proj(1)
    v1_proj(1)
    load_rc(3)
    apply1(3, stats1(3))
    k1_proj(2)
    v1_proj(2)
    k1_proj(3)
    v1_proj(3)
    q1_proj(1)

    # ============== attention helper =======================================
    def attention(q_sb, k_sb, v_sb, masked, prefix, attn):
        """Softmax attention; normalized output goes to attn (bf16)."""
        for t in range(2):
            ext = (8 * (t + 1)) if masked else NKB
            nhalf = (ext + 7) // 8
            ets = [xep.tile([P, 8 * 512], BF16, name=f"{prefix}et{t}_{i}",
                            tag="xe") for i in range(nhalf)]

            def et_sl(kb):
                return ets[kb // 8][:, (kb % 8) * 512:(kb % 8) * 512 + 512]

            for kb in range(ext):
                sp = ps_tile(f"{prefix}sp", "mm", 2)
                for eb in range(NE):
                    nc.tensor.matmul(
                        sp[:],
                        k_sb[:, eb * S + kb * P:eb * S + kb * P + P],
                        q_sb[:, eb * RO + t * 512:eb * RO + t * 512 + 512],
                        start=(eb == 0), stop=(eb == NE - 1))
                if masked and kb >= 8 * t:
                    mo = (kb - 8 * t) * 512
                    mt = st5.tile([P, 512], BF16, name=f"{prefix}mt{t}_{kb}",
                                  tag="s5")
                    nc.sync.dma_start(mt[:], d["mask"][t, :, mo:mo + 512])
                    nc.vector.tensor_add(sp[:], sp[:], mt[:])
                nc.scalar.activation(et_sl(kb), sp[:], ACT_F.Exp)
            # softmax denominator: ones-matmul column sums (pre-broadcast);
            # 1/sum is folded into the AV PSUM evacuation below
            sm = ps_tile(f"{prefix}sm", "sx", 2)
            for kb in range(ext):
                nc.tensor.matmul(sm[:], ones_1[:], et_sl(kb),
                                 start=(kb == 0), stop=(kb == ext - 1))
            inv = invp.tile([P, 512], F32, name=f"{prefix}inv{t}",
                            tag=f"i{t}")
            nc.vector.reciprocal_approx_fast(inv[:], sm[:])
            for af in range(NE):
                ap_ = ps_tile(f"{prefix}avp", "av", 2)
                for kb in range(ext):
                    nc.tensor.matmul(
                        ap_[:],
                        v_sb[:, kb * E + af * P:kb * E + af * P + P],
                        et_sl(kb),
                        start=(kb == 0), stop=(kb == ext - 1))
                o = af * RO + t * 512
                nc.vector.tensor_mul(attn[:, o:o + 512], ap_[:], inv[:])

    def o_proj_residual(attn, wo, res_getter, tag, after_rc=None):
        """h[of,rc] (bf16) = W_o.T @ attn + residual, rc-major."""
        for rc in range(2):
            for of in range(NE):
                op = ps_tile(f"{tag}op", "mm", 2)
                for ab in range(NE):
                    nc.tensor.matmul(
                        op[:],
                        wo[:, ab * E + of * P:ab * E + of * P + P],
                        attn[:, ab * RO + rc * 512:ab * RO + rc * 512 + 512],
                        start=(ab == 0), stop=(ab == NE - 1))
                o = of * RO + rc * 512
                nc.vector.tensor_add(h[:, o:o + 512], op[:],
                                     res_getter(of, rc))
            if after_rc is not None:
                after_rc(rc)

    def ln_sq(rc, prefix):
        """GpSimd squares of one owned 512-chunk of h (for LN sum(x^2))."""
        sqs = []
        for eb in range(NE):
            sl = h[:, eb * RO + rc * 512:eb * RO + rc * 512 + 512]
            sq = sq8.tile([P, 512], BF16, name=f"{prefix}sq{eb}", tag="sq")
            nc.gpsimd.tensor_mul(sq[:], sl, sl)
            sqs.append(sq)
        return sqs

    # ============== self-attention + O1 ====================================
    attn1 = qxp.tile([P, NE * RO], BF16, name="attn1", tag="qx")
    # prefetch: slots for these free as phase-A weights die
    wo1 = w_tile("wo1")
    wq2 = w_tile("wq2")
    wk2 = w_tile("wk2")

    attention(q1, k1, v1, True, "sa", attn1)

    sq2 = [None, None]

    def after_o1(rc):
        sq2[rc] = ln_sq(rc, f"l2p{rc}")

    def res1(of, rc):
        rt = st5.tile([P, 512], BF16, name=f"res{of}_{rc}", tag="s5")
        o = of * RO + rc * 512
        nc.sync.dma_start(rt[:], d["tgto"][:, o:o + 512])
        return rt[:]

    o_proj_residual(attn1, wo1, res1, "o1", after_rc=after_o1)

    # prefetch (slots free at O1 end / Q2 end)
    wv2 = w_tile("wv2")
    wo2 = w_tile("wo2")

    # ============== LN2 + K2/Q2/V2 =========================================
    xn2 = qxp.tile([P, NE * RO], BF16, name="xn2", tag="qx")
    q2 = qxp.tile([P, NE * RO], BF16, name="q2", tag="qx")
    k2 = kvp.tile([P, NE * S], BF16, name="k2", tag="kv")
    v2 = kvp.tile([P, NKB * E], BF16, name="v2", tag="kv")

    def load_src(rc, nm):
        tiles = xep.tile([P, NE * 512], BF16, name=nm, tag="xe")
        for eb in range(NE):
            nc.sync.dma_start(tiles[:, eb * 512:eb * 512 + 512],
                              d["src_t"][rc, eb])
        return tiles

    a2b2 = [stat_tiles(f"l2{rc}") for rc in range(2)]

    def stats2(rc):
        a, b = a2b2[rc]
        ln_stats(lambda eb: h[:, eb * RO + rc * 512:eb * RO + rc * 512 + 512],
                 lambda eb: sq2[rc][eb][:], a, b, f"l2s{rc}")

    def k2_proj(rc, src_rc):
        for kf in range(NE):
            kp = ps_tile("kp2", "mm", 2)
            for eb in range(NE):
                nc.tensor.matmul(
                    kp[:],
                    wk2[:, eb * E + kf * P:eb * E + kf * P + P],
                    src_rc[:, eb * 512:eb * 512 + 512],
                    start=(eb == 0), stop=(eb == NE - 1))
            nc.scalar.copy(k2[:, kf * S + rc * 512:kf * S + rc * 512 + 512],
                           kp[:])

    srcK = load_src(0, "srcK0")
    stats2(0)
    k2_proj(0, srcK)
    srcK1 = load_src(1, "srcK1")
    stats2(1)
    for rc in range(2):
        a, b = a2b2[rc]
        for eb in range(NE):
            o = eb * RO + rc * 512
            ln_apply(xn2[:, o:o + 512], h[:, o:o + 512], a, b, f"l2a{rc}")
    k2_proj(1, srcK1)
    srcK2 = load_src(2, "srcK2")
    k2_proj(2, srcK2)
    srcK3 = load_src(3, "srcK3")
    k2_proj(3, srcK3)
    # Q2 projection (owned rows)
    for fblk in range(NE):
        for rc in range(2):
            qp = ps_tile("q2p", "mm", 2)
            for eb in range(NE):
                nc.tensor.matmul(
                    qp[:],
                    wq2[:, eb * E + fblk * P:eb * E + fblk * P + P],
                    xn2[:, eb * RO + rc * 512:eb * RO + rc * 512 + 512],
                    start=(eb == 0), stop=(eb == NE - 1))
            o = fblk * RO + rc * 512
            nc.scalar.copy(q2[:, o:o + 512], qp[:])
    # V2 (re-stream src chunks)
    for rc in range(4):
        src_rc = load_src(rc, f"srcV{rc}")
        for rb in range(4):
            for vf in range(2):
                vp = ps_tile("vp2", "mm", 2)
                for eb in range(NE):
                    nc.tensor.matmul(
                        vp[:],
                        src_rc[:, eb * 512 + rb * P:eb * 512 + rb * P + P],
                        wv2[:, eb * E + vf * 512:eb * E + vf * 512 + 512],
                        start=(eb == 0), stop=(eb == NE - 1))
                o = (rc * 4 + rb) * E + vf * 512
                nc.scalar.copy(v2[:, o:o + 512], vp[:])

    # ============== cross-attention + O2 (in-place residual) ===============
    attn2 = qxp.tile([P, NE * RO], BF16, name="attn2", tag="qx")

    # w1 stream prefetch (fresh slots, DMAs run during CA)
    w1_tiles = {}

    def w1_tile(fb):
        if fb not in w1_tiles:
            t = w1p.tile([P, NE * P], BF16, name=f"w1t{fb}", tag="w1")
            nc.sync.dma_start(t[:], d["w1"][fb])
            w1_tiles[fb] = t
        return w1_tiles[fb]

    for fb in range(3):
        w1_tile(fb)

    attention(q2, k2, v2, False, "ca", attn2)

    sq3 = [None, None]

    def after_o2(rc):
        sq3[rc] = ln_sq(rc, f"l3p{rc}")

    o_proj_residual(attn2, wo2,
                    lambda of, rc: h[:, of * RO + rc * 512:
                                     of * RO + rc * 512 + 512],
                    "o2", after_rc=after_o2)

    # ============== LN3 + FFN + final residual =============================
    xn3 = qxp.tile([P, NE * RO], BF16, name="xn3", tag="qx")
    hft_a = kvp.tile([P, 16 * RO], BF16, name="hft_a", tag="kv")
    hft_b = kvp.tile([P, 16 * RO], BF16, name="hft_b", tag="kv")

    def hft_sl(fb, rc):
        t = hft_a if fb < 16 else hft_b
        o = (fb % 16) * RO + rc * 512
        return t[:, o:o + 512]

    a3b3 = [stat_tiles(f"l3{rc}") for rc in range(2)]

    def apply3(rc):
        a, b = a3b3[rc]
        for eb in range(NE):
            o = eb * RO + rc * 512
            ln_apply(xn3[:, o:o + 512], h[:, o:o + 512], a, b, f"l3a{rc}")

    for rc in range(2):
        a, b = a3b3[rc]
        ln_stats(lambda eb: h[:, eb * RO + rc * 512:eb * RO + rc * 512 + 512],
                 lambda eb: sq3[rc][eb][:], a, b, f"l3s{rc}")
        apply3(rc)

    # FF1: first rc1-groups deferred so apply3(rc1) hides behind rc0 work
    ff1_order = [(0, 0), (1, 0), (0, 1), (1, 1)] + \
        [(fb, rc) for fb in range(2, NF) for rc in range(2)]
    for fb, rc in ff1_order:
        w1t = w1_tile(fb)
        if rc == 0 and fb + 2 < NF:
            w1_tile(fb + 2)  # keep the w1 DMA stream two tiles ahead
        hps = ps_tile("hps", "mm", 2)
        for eb in range(NE):
            nc.tensor.matmul(
                hps[:],
                w1t[:, eb * P:eb * P + P],
                xn3[:, eb * RO + rc * 512:eb * RO + rc * 512 + 512],
                start=(eb == 0), stop=(eb == NE - 1))
        nc.scalar.activation(hft_sl(fb, rc), hps[:], ACT_F.Relu)

    # FF2 + final residual in fp32 + chunked output DMA
    w2_tiles = []

    def w2_prefetch(upto):
        while len(w2_tiles) < min(upto, NE):
            j = len(w2_tiles)
            t = warena.tile([P, NF * P], BF16, name=f"w2t{j}", tag="w")
            nc.sync.dma_start(t[:], d["w2"][j])
            w2_tiles.append(t)

    w2_prefetch(2)
    for of in range(NE):
        w2_prefetch(of + 3)
        w2t = w2_tiles[of]
        for rc in range(2):
            ops = ps_tile("ops", "mm", 2)
            for fb in range(NF):
                nc.tensor.matmul(
                    ops[:],
                    w2t[:, fb * P:fb * P + P],
                    hft_sl(fb, rc),
                    start=(fb == 0), stop=(fb == NF - 1))
            o = of * RO + rc * 512
            ot = outp.tile([P, 512], F32, name=f"out{of}_{rc}", tag="ot")
            nc.vector.tensor_add(ot[:], ops[:], h[:, o:o + 512])
            nc.sync.dma_start(d["out_t"][:, o:o + 512], ot[:])

    for p_ in (w1p, hpool, xep, qxp, kvp, warena, outp, st5, invp, statp,
               sq8, tmp, consts, ps):
        p_.release()


# ---------------------------------------------------------------------------
# host side: input swizzling, weight folding, output assembly
# ---------------------------------------------------------------------------

def _swz_w(w_t):
    """[E_in, N] (already [in, out]) -> SBUF image [P, (E_in/P)*N]."""
    e_in, n = w_t.shape
    return np.ascontiguousarray(
        w_t.reshape(e_in // P, P, n).transpose(1, 0, 2).reshape(P, -1))


def _own_rows(h):
    idx = []
    for j in range(8):
        g = 2 * j + h
        idx.extend(range(g * P, (g + 1) * P))
    return np.array(idx)


# swap even/odd 128-row groups: [1,0,3,2,5,4,...]
_BLKSWAP = np.arange(NKB).reshape(-1, 2)[:, ::-1].reshape(-1)


def _chunked(x_t):
    """[E, S] -> [4, NE, P, 512] (rc-chunk major, feature-block, part)."""
    return np.ascontiguousarray(
        x_t.reshape(NE, P, 4, 512).transpose(2, 0, 1, 3))


def make_in_maps(inputs):
    f32 = np.float32
    tgt = np.asarray(inputs["tgt_embs"], f32)
    src = np.asarray(inputs["src_encs"], f32)

    g1 = np.asarray(inputs["ln1_g"], f32)
    g2 = np.asarray(inputs["ln2_g"], f32)
    g3 = np.asarray(inputs["ln3_g"], f32)
    for nm in ("sa_bq", "sa_bk", "sa_bv", "sa_bo", "ca_bq", "ca_bk", "ca_bv",
               "ca_bo", "ff_b1", "ff_b2", "ln1_b", "ln2_b", "ln3_b"):
        assert np.abs(np.asarray(inputs[nm])).max() == 0.0, \
            f"nonzero bias {nm} not supported"

    scale = f32(1.0 / np.sqrt(E))
    wq1 = np.asarray(inputs["sa_Wq"], f32) * g1[None, :] * scale
    wk1 = np.asarray(inputs["sa_Wk"], f32) * g1[None, :]
    wv1 = np.asarray(inputs["sa_Wv"], f32) * g1[None, :]
    wo1 = np.asarray(inputs["sa_Wo"], f32)
    wq2 = np.asarray(inputs["ca_Wq"], f32) * g2[None, :] * scale
    wk2 = np.asarray(inputs["ca_Wk"], f32)
    wv2 = np.asarray(inputs["ca_Wv"], f32)
    wo2 = np.asarray(inputs["ca_Wo"], f32)
    w1 = np.asarray(inputs["ff_W1"], f32) * g3[None, :]
    w2 = np.asarray(inputs["ff_W2"], f32)

    w_sb = {
        "wq1": _swz_w(wq1.T.astype(BF)), "wk1": _swz_w(wk1.T.astype(BF)),
        "wv1": _swz_w(wv1.T.astype(BF)), "wo1": _swz_w(wo1.T.astype(BF)),
        "wq2": _swz_w(wq2.T.astype(BF)), "wk2": _swz_w(wk2.T.astype(BF)),
        "wv2": _swz_w(wv2.T.astype(BF)), "wo2": _swz_w(wo2.T.astype(BF)),
    }
    w1t = w1.T.astype(BF)  # [E, F]
    w1_sw = np.ascontiguousarray(
        w1t.reshape(NE, P, NF, P).transpose(2, 1, 0, 3).reshape(NF, P, NE * P))
    w2t = w2.T.astype(BF)  # [F, E]
    w2_sw = np.ascontiguousarray(
        w2t.reshape(NF, P, NE, P).transpose(2, 1, 0, 3).reshape(NE, P, NF * P))

    in_maps = []
    for c in range(NCORES):
        b, h = c // 2, c % 2
        rows = _own_rows(h)
        # perm: physical row position -> original row index (h=1 swaps each
        # even/odd 128-row group pair so owned groups land at even positions)
        if h == 1:
            perm = (_BLKSWAP[:, None] * P + np.arange(P)[None, :]).reshape(-1)
        else:
            perm = np.arange(S)
        tgt_t = _chunked(tgt[b][perm].T).astype(BF)
        tgto = _swz_w(np.ascontiguousarray(tgt[b][rows].T)).astype(BF)
        src_t = _chunked(src[b].T).astype(BF)
        mask = np.zeros((2, 8, P, 512), np.float32)
        for t in range(2):
            kr = perm[1024 * t:1024 * t + 1024]  # original index of each key
            qg = np.empty(512, np.int64)
            for s in range(4):
                g = 8 * t + 2 * s + h
                qg[s * P:(s + 1) * P] = g * P + np.arange(P)
            m = np.where(kr[:, None] <= qg[None, :], 0.0, NEG).astype(np.float32)
            mask[t] = m.reshape(8, P, 512)
        # kernel mask layout: [t, P, kb*512]
        mask_k = np.ascontiguousarray(mask.transpose(0, 2, 1, 3)
                                      .reshape(2, P, 8 * 512))
        in_maps.append({
            "tgt_t": tgt_t,
            "tgto": tgto,
            "src_t": src_t,
            "mask": mask_k.astype(BF),
            **w_sb,
            "w1": w1_sw,
            "w2": w2_sw,
        })
    return in_maps


def assemble_output(results):
    out = np.empty((B, S, E), np.float32)
    for c in range(NCORES):
        b, h = c // 2, c % 2
        arr = np.asarray(results[c]["out_t"])  # [P, NE*RO]
        a = arr.reshape(P, NE, 8, P).transpose(2, 3, 1, 0).reshape(8, P, E)
        for j in range(8):
            g = 2 * j + h
            out[b, g * P:(g + 1) * P, :] = a[j]
    return out


def get_nc():
    if "nc" not in _NC_CACHE:
        _NC_CACHE["nc"] = _build_program()
    return _NC_CACHE["nc"]


def _axon_reset():
    """Recover a wedged remote NeuronCore (NRT_EXEC_UNIT_UNRECOVERABLE)."""
    try:
        import ctypes
        lib = ctypes.CDLL("/opt/axon/libaxon_pjrt.so")
        lib.axon_reset.restype = ctypes.c_int64
        lib.axon_reset()
    except Exception:
        pass


def kernel(**inputs):
    global LAST_RESULTS
    in_maps = make_in_maps(inputs)
    nc = get_nc()
    last_err = None
    for attempt in range(3):
        try:
            res = run_bass_kernel_spmd(nc, in_maps, list(range(NCORES)))
            break
        except Exception as e:  # wedged device -> reset + retry
            last_err = e
            _axon_reset()
    else:
        raise last_err
    LAST_RESULTS = res
    return assemble_output(res.results)


# revision 46
# speedup vs baseline: 1.1500x; 1.1500x over previous
"""Trainium2 Bass kernel for a single-head transformer decoder layer.

Model (per batch element, S=2048, E=1024, F=4096):
    xn  = LN(tgt);  sa = causal_attn(xn)       ; h   = tgt + sa
    xn2 = LN(h);    ca = cross_attn(xn2, src)  ; h  += ca
    xn3 = LN(h);    ff = relu(xn3@W1.T)@W2.T   ; out = h + ff

Sharding: 8 cores = 4 batches x 2-way query-row split.  Core c owns batch
b=c//2 and interleaved 128-row chunks g = 2*j + (c%2), j=0..7 (zig-zag, so
causal-attention work is balanced across the pair).  K/V projections over
all 2048 rows are duplicated within each pair; no collectives.

On-chip layout: activations are stored transposed [feature(part), row(free)],
which lets every matmul in the layer run without any on-chip transpose:
  - proj:    out_T[f,r]   = mm(lhsT=W_T[e,f] blk,  rhs=x_T[e,r])
  - V:       V_nat[r,v]   = mm(lhsT=x_T[e,r] blk,  rhs=W_T[e,v])
  - scores:  s_T[kr,qr]   = mm(lhsT=K_T[e,kr] blk, rhs=Q_T[e,qr])
  - softmax: exp in-place (no max-sub needed; scores are O(1)), column sums
             via ones-matmul (M=128 -> pre-broadcast), causal mask as
             additive bf16 input data
  - AV:      a_T[af,qr]   = mm(lhsT=V_nat[kr,af] blk, rhs=expT[kr,qr])
  - the softmax denominator is divided out after the O-projection, fused
    into the residual add
LN gain and the 1/sqrt(E) score scale are folded into the projection
weights on the host (exact); all biases in this problem are zero.  Q1 is
projected straight from the full-row LN1 output via a strided rhs AP (each
core's owned rows are pre-swizzled to the even 128-col blocks of every
512-chunk), so no separate owned-row LN pass is needed.

Memory: SBUF is managed as a handful of program-long pools whose tags act
as free-slot rings; successive logical tensors (k1->k2->hft, q1->attn1->
xn2->q2->attn2->xn3, the eight 2MB weight images + the w2 stream, the
xn/exp/src-chunk 1MB tiles) reuse slots with WAR deps that naturally time
each prefetch DMA right when its slot's last reader finishes.  This keeps
every large DMA at least one phase ahead of its consumer, which is what
keeps the PE from ever going idle (and from HAM-rethrottling).

Numerics: matmul operands bf16; PSUM accumulation, LN stats and softmax
sums stay fp32.  The residual stream h is bf16 in SBUF (two bf16 roundings
of an O(1) stream, well inside the 2e-2 budget); the final residual add
runs in fp32 and the output is fp32.
"""

import os
import sys

import numpy as np

for _p in ("/opt/trn_rl_repo", "/root/.axon_site/_ro/trn_rl_repo"):
    if os.path.isdir(_p) and _p not in sys.path:
        sys.path.insert(0, _p)

import ml_dtypes  # noqa: E402

import concourse.bass as bass  # noqa: E402
import concourse.tile as tile  # noqa: E402
from concourse import bacc, mybir  # noqa: E402
from concourse.bass_utils import run_bass_kernel_spmd  # noqa: E402

E = 1024
S = 2048
B = 4
F = 4096
P = 128
NE = E // P          # 8 feature blocks
NF = F // P          # 32 ff blocks
NKB = S // P         # 16 key-row blocks
RO = 1024            # owned query rows per core
NCORES = 8

F32 = mybir.dt.float32
BF16 = mybir.dt.bfloat16
F8 = mybir.dt.float8e4
BF = ml_dtypes.bfloat16
F8NP = ml_dtypes.float8_e4m3
ALU = mybir.AluOpType
ACT_F = mybir.ActivationFunctionType
DR = mybir.MatmulPerfMode.DoubleRow
WS = 64.0     # host-side fp8 weight scale (2^6, exact); undone at evacuation
IS = 1.0 / WS

NEG = -1e30

_NC_CACHE = {}
LAST_RESULTS = None  # BassKernelResults of the most recent hardware run


def _build_program():
    """Emit the single SPMD program (identical for all 8 cores)."""
    nc = bacc.Bacc(
        "TRN2",
        target_bir_lowering=False,
        debug=False,
        enable_asserts=False,
        num_devices=NCORES,
    )

    d = {}
    d["tgt_t"] = nc.dram_tensor("tgt_t", [4, NE, P, 512], BF16, kind="ExternalInput")
    d["tgto"] = nc.dram_tensor("tgto", [P, NE * RO], BF16, kind="ExternalInput")
    d["src_t"] = nc.dram_tensor("src_t", [4, NE, P, 512], F8, kind="ExternalInput")
    d["mask"] = nc.dram_tensor("mask", [2, P, 8 * 512], BF16, kind="ExternalInput")
    for w in ("wq1", "wk1", "wv1", "wq2", "wk2", "wv2"):
        d[w] = nc.dram_tensor(w, [P, NE * E], F8, kind="ExternalInput")
    for w in ("wo1", "wo2"):
        d[w] = nc.dram_tensor(w, [P, NE * E], BF16, kind="ExternalInput")
    d["w1"] = nc.dram_tensor("w1", [NF, P, NE * P], BF16, kind="ExternalInput")
    d["w2"] = nc.dram_tensor("w2", [NE, P, NF * P], BF16, kind="ExternalInput")
    d["out_t"] = nc.dram_tensor("out_t", [P, NE * RO], F32, kind="ExternalOutput")

    with tile.TileContext(nc) as tc:
        with nc.allow_low_precision(
                reason="bf16 LN stats / softmax inv are within the 2e-2 "
                       "relative-error budget (validated in sim)"):
            _emit(tc, {k: v.ap() for k, v in d.items()})

    nc.compile()
    return nc


def _emit(tc, d):
    nc = tc.nc

    # --- PSUM: one pool, 8 banks total across tags -------------------------
    ps = tc.alloc_tile_pool(name="ps", bufs=1, space="PSUM")

    def ps_tile(name, tag, bufs, shape=(P, 512)):
        return ps.tile(list(shape), F32, name=name, tag=tag, bufs=bufs)

    # --- SBUF: program-long pools; tags are free-slot rings ----------------
    def pool(name, bufs=1, side="left"):
        return tc.alloc_tile_pool(name=name, bufs=bufs, side=side)

    consts = pool("consts")
    ones_1 = consts.tile([P, P], BF16, name="ones_1", tag="ones_1")
    nc.vector.memset(ones_1[:], 1.0)
    # 1/E (= 2^-10, exact in bf16) folded into the LN stat sums
    ones_m = consts.tile([P, P], BF16, name="ones_m", tag="ones_m")
    nc.vector.memset(ones_m[:], 1.0 / E)
    eps_t = consts.tile([P, 1], F32, name="eps_t", tag="eps")
    nc.vector.memset(eps_t[:], 1e-5)

    tmp = pool("tmp", bufs=1)        # LN chain scratch, tags t0/t1 (f32)
    sq8 = pool("sq8", bufs=4)        # bf16 squares (DVE) for LN sum(x^2)
    statp = pool("statp", bufs=2)    # LN A/B stat tiles (bf16, 2 rc in flight)
    invp = pool("invp", bufs=1)      # softmax 1/sum tiles (bf16, tags i0/i1)
    st5 = pool("st5", bufs=8)        # [P,512] bf16 stream: tgt-in/mask/res
    outp = pool("outp", bufs=1)      # [P,512] f32 output staging
    warena = pool("warena", bufs=3)  # 2MB slots: 8 proj weights + w2 stream
    kvp = pool("kvp", bufs=2)        # 4MB slots: k1,v1 -> k2,v2 -> hft a/b
    qxp = pool("qxp", bufs=2)        # 2MB slots: q1,attn1 -> xn2,q2 -> attn2,xn3
    xep = pool("xep", bufs=2)        # 1MB slots: xn chunks, exp tiles, src chunks
    hpool = pool("hpool")            # residual stream h (bf16, 2MB)
    w1p = pool("w1p", bufs=3)        # 256KB slots: w1 stream

    h = hpool.tile([P, NE * RO], BF16, name="h", tag="h")

    def w_tile(nm, dt=F8):
        t = warena.tile([P, NE * E], dt, name=nm, tag="w")
        nc.sync.dma_start(t[:], d[nm][:])
        return t

    def wpair(w, i, c0, cw):
        """[P,2,cw] k-block pair (2i,2i+1) of a [P, NE*E] weight image."""
        return w[:].rearrange("p (e c) -> p e c", e=NE)[:, 2 * i:2 * i + 2,
                                                       c0:c0 + cw]

    def ln_stats(get_x, get_sq, a_tile, b_tile, prefix):
        """Per-row LN stats over one transposed 512-chunk.

        get_x(eb) -> [P,512] bf16 AP; get_sq(eb) -> [P,512] bf16 AP of x^2.
        Fills a_tile = rsqrt(var+eps), b_tile = mean * a  (bf16, broadcast
        along partitions by the 1/E-scaled ones-matmul).
        """
        mu = ps_tile(f"{prefix}sx", "sx", 2)     # mean (1/E in ones_m)
        ex2 = ps_tile(f"{prefix}sxx", "sxx", 2)  # E[x^2]
        for eb in range(NE):
            x = get_x(eb)
            sq = get_sq(eb)
            nc.tensor.matmul(mu[:], ones_m[:], x,
                             start=(eb == 0), stop=(eb == NE - 1))
            nc.tensor.matmul(ex2[:], ones_m[:], sq,
                             start=(eb == 0), stop=(eb == NE - 1))
        v = tmp.tile([P, 512], F32, name=f"{prefix}v", tag="t1")
        nc.scalar.square(v[:], mu[:])
        nc.vector.scalar_tensor_tensor(
            v[:], ex2[:], 1.0, v[:], ALU.mult, ALU.subtract)
        nc.scalar.activation(v[:], v[:], ACT_F.Sqrt, bias=eps_t[:])
        nc.vector.reciprocal_approx_fast(v[:], v[:])
        nc.scalar.copy(a_tile[:], v[:])
        nc.vector.tensor_mul(b_tile[:], mu[:], a_tile[:])

    def ln_apply(dst, src_ap, a, bv, prefix):
        """dst (bf16) = src*A - Bv (bf16 throughout for DVE fast modes)."""
        t = tmp.tile([P, 512], BF16, name=f"{prefix}ap", tag="t0")
        nc.vector.tensor_mul(t[:], src_ap, a[:])
        nc.vector.tensor_sub(dst, t[:], bv[:])

    def stat_tiles(nm):
        a = statp.tile([P, 512], BF16, name=f"{nm}A", tag="A")
        b = statp.tile([P, 512], BF16, name=f"{nm}B", tag="B")
        return a, b

    # ============== phase A: LN1 over all rows -> K1, V1, Q1 ===============
    k1 = kvp.tile([P, NE * S], BF16, name="k1", tag="kv")
    v1 = kvp.tile([P, NKB * E], BF16, name="v1", tag="kv")
    q1 = qxp.tile([P, NE * RO], BF16, name="q1", tag="qx")

    T = [None] * 4

    def load_rc(rc):
        tiles = []
        for eb in range(NE):
            t = st5.tile([P, 512], BF16, name=f"tgt{rc}_{eb}", tag="s5")
            nc.sync.dma_start(t[:], d["tgt_t"][rc, eb])
            tiles.append(t)
        T[rc] = tiles

    # DMA order: first stats chunk, then the phase-A weights
    load_rc(0)
    wk1 = w_tile("wk1")
    load_rc(1)
    wv1 = w_tile("wv1")
    wq1 = w_tile("wq1")

    def stats1(rc):
        sqs = []
        for eb in range(NE):
            sq = sq8.tile([P, 512], BF16, name=f"l1sq{rc}_{eb}", tag="sq")
            nc.gpsimd.tensor_mul(sq[:], T[rc][eb][:], T[rc][eb][:])
            sqs.append(sq)
        a, b = stat_tiles(f"l1f{rc}")
        ln_stats(lambda eb: T[rc][eb][:], lambda eb: sqs[eb][:], a, b,
                 f"l1f{rc}")
        return a, b

    xn = [None] * 4

    def apply1(rc, ab):
        a, b = ab
        x = xep.tile([P, NE * 512], F8, name=f"xn{rc}", tag="xe")
        for eb in range(NE):
            ln_apply(x[:, eb * 512:eb * 512 + 512], T[rc][eb][:], a, b,
                     f"l1a{rc}")
        xn[rc] = x

    def xnpair(x, i, c0, cw):
        return x[:].rearrange("p (e c) -> p e c", e=NE)[:, 2 * i:2 * i + 2,
                                                        c0:c0 + cw]

    def k1_proj(rc):
        for kf in range(NE):
            kp = ps_tile("kp", "mm", 2)
            for i in range(NE // 2):
                nc.tensor.matmul(
                    kp[:], wpair(wk1, i, kf * P, P), xnpair(xn[rc], i, 0, 512),
                    start=(i == 0), stop=(i == NE // 2 - 1), perf_mode=DR)
            nc.scalar.activation(
                k1[:, kf * S + rc * 512:kf * S + rc * 512 + 512], kp[:],
                ACT_F.Copy, scale=IS)

    def v1_proj(rc):
        for rb in range(4):
            for vf in range(2):
                vp = ps_tile("vp", "mm", 2)
                for i in range(NE // 2):
                    nc.tensor.matmul(
                        vp[:], xnpair(xn[rc], i, rb * P, P),
                        wpair(wv1, i, vf * 512, 512),
                        start=(i == 0), stop=(i == NE // 2 - 1), perf_mode=DR)
                o = (rc * 4 + rb) * E + vf * 512
                nc.scalar.activation(v1[:, o:o + 512], vp[:], ACT_F.Copy,
                                     scale=IS)

    def q1_proj(c):
        # own chunk c (512 cols) = the even 128-col blocks of xn[2c],
        # xn[2c+1] (the host pre-swizzles tgt so each core's owned rows land
        # at even block positions; mask/tgto follow the swizzle).
        for half in range(2):
            rc = 2 * c + half
            xv = xn[rc][:].rearrange("p (e b t c) -> p e t b c",
                                     e=NE, b=2, t=2, c=P)
            for fblk in range(NE):
                qp = ps_tile("qp", "mm", 2, shape=(P, 256))
                for eb in range(NE):
                    nc.tensor.matmul(
                        qp[:],
                        wq1[:, eb * E + fblk * P:eb * E + fblk * P + P],
                        xv[:, eb, 0],
                        start=(eb == 0), stop=(eb == NE - 1))
                o = fblk * RO + c * 512 + half * 256
                nc.scalar.activation(q1[:, o:o + 256], qp[:], ACT_F.Copy,
                                     scale=IS)

    apply1(0, stats1(0))
    apply1(1, stats1(1))
    k1_proj(0)
    v1_proj(0)
    q1_proj(0)
    load_rc(2)
    apply1(2, stats1(2))
    k1_proj(1)
    v1_proj(1)
    load_rc(3)
    apply1(3, stats1(3))
    k1_proj(2)
    v1_proj(2)
    k1_proj(3)
    v1_proj(3)
    q1_proj(1)

    # ============== attention helper =======================================
    def attention(q_sb, k_sb, v_sb, masked, prefix, attn):
        """Softmax attention; normalized output goes to attn (bf16)."""
        for t in range(2):
            ext = (8 * (t + 1)) if masked else NKB
            nhalf = (ext + 7) // 8
            ets = [xep.tile([P, 8 * 512], BF16, name=f"{prefix}et{t}_{i}",
                            tag="xe") for i in range(nhalf)]

            def et_sl(kb):
                return ets[kb // 8][:, (kb % 8) * 512:(kb % 8) * 512 + 512]

            for kb in range(ext):
                sp = ps_tile(f"{prefix}sp", "mm", 2)
                for eb in range(NE):
                    nc.tensor.matmul(
                        sp[:],
                        k_sb[:, eb * S + kb * P:eb * S + kb * P + P],
                        q_sb[:, eb * RO + t * 512:eb * RO + t * 512 + 512],
                        start=(eb == 0), stop=(eb == NE - 1))
                if masked and kb >= 8 * t:
                    mo = (kb - 8 * t) * 512
                    mt = st5.tile([P, 512], BF16, name=f"{prefix}mt{t}_{kb}",
                                  tag="s5")
                    nc.sync.dma_start(mt[:], d["mask"][t, :, mo:mo + 512])
                    nc.vector.tensor_add(sp[:], sp[:], mt[:])
                nc.scalar.activation(et_sl(kb), sp[:], ACT_F.Exp)
            # softmax denominator: ones-matmul column sums (pre-broadcast);
            # 1/sum is folded into the AV PSUM evacuation below
            sm = ps_tile(f"{prefix}sm", "sx", 2)
            for kb in range(ext):
                nc.tensor.matmul(sm[:], ones_1[:], et_sl(kb),
                                 start=(kb == 0), stop=(kb == ext - 1))
            inv = invp.tile([P, 512], F32, name=f"{prefix}inv{t}",
                            tag=f"i{t}")
            nc.vector.reciprocal_approx_fast(inv[:], sm[:])
            for af in range(NE):
                ap_ = ps_tile(f"{prefix}avp", "av", 2)
                for kb in range(ext):
                    nc.tensor.matmul(
                        ap_[:],
                        v_sb[:, kb * E + af * P:kb * E + af * P + P],
                        et_sl(kb),
                        start=(kb == 0), stop=(kb == ext - 1))
                o = af * RO + t * 512
                nc.vector.tensor_mul(attn[:, o:o + 512], ap_[:], inv[:])

    def o_proj_residual(attn, wo, res_getter, tag, after_rc=None):
        """h[of,rc] (bf16) = W_o.T @ attn + residual, rc-major."""
        for rc in range(2):
            for of in range(NE):
                op = ps_tile(f"{tag}op", "mm", 2)
                for ab in range(NE):
                    nc.tensor.matmul(
                        op[:],
                        wo[:, ab * E + of * P:ab * E + of * P + P],
                        attn[:, ab * RO + rc * 512:ab * RO + rc * 512 + 512],
                        start=(ab == 0), stop=(ab == NE - 1))
                o = of * RO + rc * 512
                nc.vector.tensor_add(h[:, o:o + 512], op[:],
                                     res_getter(of, rc))
            if after_rc is not None:
                after_rc(rc)

    def ln_sq(rc, prefix):
        """GpSimd squares of one owned 512-chunk of h (for LN sum(x^2))."""
        sqs = []
        for eb in range(NE):
            sl = h[:, eb * RO + rc * 512:eb * RO + rc * 512 + 512]
            sq = sq8.tile([P, 512], BF16, name=f"{prefix}sq{eb}", tag="sq")
            nc.gpsimd.tensor_mul(sq[:], sl, sl)
            sqs.append(sq)
        return sqs

    # ============== self-attention + O1 ====================================
    attn1 = qxp.tile([P, NE * RO], BF16, name="attn1", tag="qx")
    # prefetch: slots for these free as phase-A weights die
    wo1 = w_tile("wo1", BF16)
    wq2 = w_tile("wq2")
    wk2 = w_tile("wk2")

    attention(q1, k1, v1, True, "sa", attn1)

    sq2 = [None, None]

    def after_o1(rc):
        sq2[rc] = ln_sq(rc, f"l2p{rc}")

    def res1(of, rc):
        rt = st5.tile([P, 512], BF16, name=f"res{of}_{rc}", tag="s5")
        o = of * RO + rc * 512
        nc.sync.dma_start(rt[:], d["tgto"][:, o:o + 512])
        return rt[:]

    o_proj_residual(attn1, wo1, res1, "o1", after_rc=after_o1)

    # prefetch (slots free at O1 end / Q2 end)
    wv2 = w_tile("wv2")
    wo2 = w_tile("wo2", BF16)

    # ============== LN2 + K2/Q2/V2 =========================================
    xn2 = qxp.tile([P, NE * RO], F8, name="xn2", tag="qx")
    q2 = qxp.tile([P, NE * RO], BF16, name="q2", tag="qx")
    k2 = kvp.tile([P, NE * S], BF16, name="k2", tag="kv")
    v2 = kvp.tile([P, NKB * E], BF16, name="v2", tag="kv")

    def load_src(rc, nm):
        tiles = xep.tile([P, NE * 512], F8, name=nm, tag="xe")
        for eb in range(NE):
            nc.sync.dma_start(tiles[:, eb * 512:eb * 512 + 512],
                              d["src_t"][rc, eb])
        return tiles

    a2b2 = [stat_tiles(f"l2{rc}") for rc in range(2)]

    def stats2(rc):
        a, b = a2b2[rc]
        ln_stats(lambda eb: h[:, eb * RO + rc * 512:eb * RO + rc * 512 + 512],
                 lambda eb: sq2[rc][eb][:], a, b, f"l2s{rc}")

    def k2_proj(rc, src_rc):
        for kf in range(NE):
            kp = ps_tile("kp2", "mm", 2)
            for i in range(NE // 2):
                nc.tensor.matmul(
                    kp[:], wpair(wk2, i, kf * P, P), xnpair(src_rc, i, 0, 512),
                    start=(i == 0), stop=(i == NE // 2 - 1), perf_mode=DR)
            nc.scalar.activation(
                k2[:, kf * S + rc * 512:kf * S + rc * 512 + 512], kp[:],
                ACT_F.Copy, scale=IS)

    srcK = load_src(0, "srcK0")
    stats2(0)
    k2_proj(0, srcK)
    srcK1 = load_src(1, "srcK1")
    stats2(1)
    for rc in range(2):
        a, b = a2b2[rc]
        for eb in range(NE):
            o = eb * RO + rc * 512
            ln_apply(xn2[:, o:o + 512], h[:, o:o + 512], a, b, f"l2a{rc}")
    k2_proj(1, srcK1)
    srcK2 = load_src(2, "srcK2")
    k2_proj(2, srcK2)
    srcK3 = load_src(3, "srcK3")
    k2_proj(3, srcK3)
    # Q2 projection (owned rows)
    xn2r = xn2[:].rearrange("p (e c) -> p e c", e=NE)
    for fblk in range(NE):
        for rc in range(2):
            qp = ps_tile("q2p", "mm", 2)
            for i in range(NE // 2):
                nc.tensor.matmul(
                    qp[:], wpair(wq2, i, fblk * P, P),
                    xn2r[:, 2 * i:2 * i + 2, rc * 512:rc * 512 + 512],
                    start=(i == 0), stop=(i == NE // 2 - 1), perf_mode=DR)
            o = fblk * RO + rc * 512
            nc.scalar.activation(q2[:, o:o + 512], qp[:], ACT_F.Copy,
                                 scale=IS)
    # V2 (re-stream src chunks)
    for rc in range(4):
        src_rc = load_src(rc, f"srcV{rc}")
        for rb in range(4):
            for vf in range(2):
                vp = ps_tile("vp2", "mm", 2)
                for i in range(NE // 2):
                    nc.tensor.matmul(
                        vp[:], xnpair(src_rc, i, rb * P, P),
                        wpair(wv2, i, vf * 512, 512),
                        start=(i == 0), stop=(i == NE // 2 - 1), perf_mode=DR)
                o = (rc * 4 + rb) * E + vf * 512
                nc.scalar.activation(v2[:, o:o + 512], vp[:], ACT_F.Copy,
                                     scale=IS)

    # ============== cross-attention + O2 (in-place residual) ===============
    attn2 = qxp.tile([P, NE * RO], BF16, name="attn2", tag="qx")

    # w1 stream prefetch (fresh slots, DMAs run during CA)
    w1_tiles = {}

    def w1_tile(fb):
        if fb not in w1_tiles:
            t = w1p.tile([P, NE * P], BF16, name=f"w1t{fb}", tag="w1")
            nc.sync.dma_start(t[:], d["w1"][fb])
            w1_tiles[fb] = t
        return w1_tiles[fb]

    for fb in range(3):
        w1_tile(fb)

    attention(q2, k2, v2, False, "ca", attn2)

    sq3 = [None, None]

    def after_o2(rc):
        sq3[rc] = ln_sq(rc, f"l3p{rc}")

    o_proj_residual(attn2, wo2,
                    lambda of, rc: h[:, of * RO + rc * 512:
                                     of * RO + rc * 512 + 512],
                    "o2", after_rc=after_o2)

    # ============== LN3 + FFN + final residual =============================
    xn3 = qxp.tile([P, NE * RO], BF16, name="xn3", tag="qx")
    hft_a = kvp.tile([P, 16 * RO], BF16, name="hft_a", tag="kv")
    hft_b = kvp.tile([P, 16 * RO], BF16, name="hft_b", tag="kv")

    def hft_sl(fb, rc):
        t = hft_a if fb < 16 else hft_b
        o = (fb % 16) * RO + rc * 512
        return t[:, o:o + 512]

    def hft_pair(j, rc):
        """[P,2,512] fb-block pair (2j,2j+1) of hft."""
        t = hft_a if 2 * j < 16 else hft_b
        r = t[:].rearrange("p (f c) -> p f c", f=16)
        jj = (2 * j) % 16
        return r[:, jj:jj + 2, rc * 512:rc * 512 + 512]

    a3b3 = [stat_tiles(f"l3{rc}") for rc in range(2)]

    def apply3(rc):
        a, b = a3b3[rc]
        for eb in range(NE):
            o = eb * RO + rc * 512
            ln_apply(xn3[:, o:o + 512], h[:, o:o + 512], a, b, f"l3a{rc}")

    for rc in range(2):
        a, b = a3b3[rc]
        ln_stats(lambda eb: h[:, eb * RO + rc * 512:eb * RO + rc * 512 + 512],
                 lambda eb: sq3[rc][eb][:], a, b, f"l3s{rc}")
        apply3(rc)

    # FF1: first rc1-groups deferred so apply3(rc1) hides behind rc0 work
    ff1_order = [(0, 0), (1, 0), (0, 1), (1, 1)] + \
        [(fb, rc) for fb in range(2, NF) for rc in range(2)]
    xn3r = xn3[:].rearrange("p (e c) -> p e c", e=NE)
    for fb, rc in ff1_order:
        w1t = w1_tile(fb)
        if rc == 0 and fb + 2 < NF:
            w1_tile(fb + 2)  # keep the w1 DMA stream two tiles ahead
        hps = ps_tile("hps", "mm", 2)
        for eb in range(NE):
            nc.tensor.matmul(
                hps[:],
                w1t[:, eb * P:eb * P + P],
                xn3[:, eb * RO + rc * 512:eb * RO + rc * 512 + 512],
                start=(eb == 0), stop=(eb == NE - 1))
        nc.scalar.activation(hft_sl(fb, rc), hps[:], ACT_F.Relu)

    # FF2 + final residual in fp32 + chunked output DMA
    w2_tiles = []

    def w2_prefetch(upto):
        while len(w2_tiles) < min(upto, NE):
            j = len(w2_tiles)
            t = warena.tile([P, NF * P], BF16, name=f"w2t{j}", tag="w")
            nc.sync.dma_start(t[:], d["w2"][j])
            w2_tiles.append(t)

    w2_prefetch(2)
    for of in range(NE):
        w2_prefetch(of + 3)
        w2t = w2_tiles[of]
        for rc in range(2):
            ops = ps_tile("ops", "mm", 2)
            for fb in range(NF):
                nc.tensor.matmul(
                    ops[:],
                    w2t[:, fb * P:fb * P + P],
                    hft_sl(fb, rc),
                    start=(fb == 0), stop=(fb == NF - 1))
            o = of * RO + rc * 512
            ot = outp.tile([P, 512], F32, name=f"out{of}_{rc}", tag="ot")
            nc.vector.tensor_add(ot[:], ops[:], h[:, o:o + 512])
            nc.sync.dma_start(d["out_t"][:, o:o + 512], ot[:])

    for p_ in (w1p, hpool, xep, qxp, kvp, warena, outp, st5, invp, statp,
               sq8, tmp, consts, ps):
        p_.release()


# ---------------------------------------------------------------------------
# host side: input swizzling, weight folding, output assembly
# ---------------------------------------------------------------------------

def _swz_w(w_t):
    """[E_in, N] (already [in, out]) -> SBUF image [P, (E_in/P)*N]."""
    e_in, n = w_t.shape
    return np.ascontiguousarray(
        w_t.reshape(e_in // P, P, n).transpose(1, 0, 2).reshape(P, -1))


def _own_rows(h):
    idx = []
    for j in range(8):
        g = 2 * j + h
        idx.extend(range(g * P, (g + 1) * P))
    return np.array(idx)


# swap even/odd 128-row groups: [1,0,3,2,5,4,...]
_BLKSWAP = np.arange(NKB).reshape(-1, 2)[:, ::-1].reshape(-1)


def _chunked(x_t):
    """[E, S] -> [4, NE, P, 512] (rc-chunk major, feature-block, part)."""
    return np.ascontiguousarray(
        x_t.reshape(NE, P, 4, 512).transpose(2, 0, 1, 3))


def make_in_maps(inputs):
    f32 = np.float32
    tgt = np.asarray(inputs["tgt_embs"], f32)
    src = np.asarray(inputs["src_encs"], f32)

    g1 = np.asarray(inputs["ln1_g"], f32)
    g2 = np.asarray(inputs["ln2_g"], f32)
    g3 = np.asarray(inputs["ln3_g"], f32)
    for nm in ("sa_bq", "sa_bk", "sa_bv", "sa_bo", "ca_bq", "ca_bk", "ca_bv",
               "ca_bo", "ff_b1", "ff_b2", "ln1_b", "ln2_b", "ln3_b"):
        assert np.abs(np.asarray(inputs[nm])).max() == 0.0, \
            f"nonzero bias {nm} not supported"

    scale = f32(1.0 / np.sqrt(E))
    wq1 = np.asarray(inputs["sa_Wq"], f32) * g1[None, :] * scale
    wk1 = np.asarray(inputs["sa_Wk"], f32) * g1[None, :]
    wv1 = np.asarray(inputs["sa_Wv"], f32) * g1[None, :]
    wo1 = np.asarray(inputs["sa_Wo"], f32)
    wq2 = np.asarray(inputs["ca_Wq"], f32) * g2[None, :] * scale
    wk2 = np.asarray(inputs["ca_Wk"], f32)
    wv2 = np.asarray(inputs["ca_Wv"], f32)
    wo2 = np.asarray(inputs["ca_Wo"], f32)
    w1 = np.asarray(inputs["ff_W1"], f32) * g3[None, :]
    w2 = np.asarray(inputs["ff_W2"], f32)

    ws = np.float32(WS)
    w_sb = {
        "wq1": _swz_w((wq1.T * ws).astype(F8NP)),
        "wk1": _swz_w((wk1.T * ws).astype(F8NP)),
        "wv1": _swz_w((wv1.T * ws).astype(F8NP)),
        "wo1": _swz_w(wo1.T.astype(BF)),
        "wq2": _swz_w((wq2.T * ws).astype(F8NP)),
        "wk2": _swz_w((wk2.T * ws).astype(F8NP)),
        "wv2": _swz_w((wv2.T * ws).astype(F8NP)),
        "wo2": _swz_w(wo2.T.astype(BF)),
    }
    w1t = w1.T.astype(BF)  # [E, F]
    w1_sw = np.ascontiguousarray(
        w1t.reshape(NE, P, NF, P).transpose(2, 1, 0, 3).reshape(NF, P, NE * P))
    w2t = w2.T.astype(BF)  # [F, E]
    w2_sw = np.ascontiguousarray(
        w2t.reshape(NF, P, NE, P).transpose(2, 1, 0, 3).reshape(NE, P, NF * P))

    in_maps = []
    for c in range(NCORES):
        b, h = c // 2, c % 2
        rows = _own_rows(h)
        # perm: physical row position -> original row index (h=1 swaps each
        # even/odd 128-row group pair so owned groups land at even positions)
        if h == 1:
            perm = (_BLKSWAP[:, None] * P + np.arange(P)[None, :]).reshape(-1)
        else:
            perm = np.arange(S)
        tgt_t = _chunked(tgt[b][perm].T).astype(BF)
        tgto = _swz_w(np.ascontiguousarray(tgt[b][rows].T)).astype(BF)
        src_t = _chunked(src[b].T).astype(F8NP)
        mask = np.zeros((2, 8, P, 512), np.float32)
        for t in range(2):
            kr = perm[1024 * t:1024 * t + 1024]  # original index of each key
            qg = np.empty(512, np.int64)
            for s in range(4):
                g = 8 * t + 2 * s + h
                qg[s * P:(s + 1) * P] = g * P + np.arange(P)
            m = np.where(kr[:, None] <= qg[None, :], 0.0, NEG).astype(np.float32)
            mask[t] = m.reshape(8, P, 512)
        # kernel mask layout: [t, P, kb*512]
        mask_k = np.ascontiguousarray(mask.transpose(0, 2, 1, 3)
                                      .reshape(2, P, 8 * 512))
        in_maps.append({
            "tgt_t": tgt_t,
            "tgto": tgto,
            "src_t": src_t,
            "mask": mask_k.astype(BF),
            **w_sb,
            "w1": w1_sw,
            "w2": w2_sw,
        })
    return in_maps


def assemble_output(results):
    out = np.empty((B, S, E), np.float32)
    for c in range(NCORES):
        b, h = c // 2, c % 2
        arr = np.asarray(results[c]["out_t"])  # [P, NE*RO]
        a = arr.reshape(P, NE, 8, P).transpose(2, 3, 1, 0).reshape(8, P, E)
        for j in range(8):
            g = 2 * j + h
            out[b, g * P:(g + 1) * P, :] = a[j]
    return out


def get_nc():
    if "nc" not in _NC_CACHE:
        _NC_CACHE["nc"] = _build_program()
    return _NC_CACHE["nc"]


def _axon_reset():
    """Recover a wedged remote NeuronCore (NRT_EXEC_UNIT_UNRECOVERABLE)."""
    try:
        import ctypes
        lib = ctypes.CDLL("/opt/axon/libaxon_pjrt.so")
        lib.axon_reset.restype = ctypes.c_int64
        lib.axon_reset()
    except Exception:
        pass


def kernel(**inputs):
    global LAST_RESULTS
    in_maps = make_in_maps(inputs)
    nc = get_nc()
    last_err = None
    for attempt in range(3):
        try:
            res = run_bass_kernel_spmd(nc, in_maps, list(range(NCORES)))
            break
        except Exception as e:  # wedged device -> reset + retry
            last_err = e
            _axon_reset()
    else:
        raise last_err
    LAST_RESULTS = res
    return assemble_output(res.results)


# revision 48
# speedup vs baseline: 1.1539x; 1.0034x over previous
"""Trainium2 Bass kernel for a single-head transformer decoder layer.

Model (per batch element, S=2048, E=1024, F=4096):
    xn  = LN(tgt);  sa = causal_attn(xn)       ; h   = tgt + sa
    xn2 = LN(h);    ca = cross_attn(xn2, src)  ; h  += ca
    xn3 = LN(h);    ff = relu(xn3@W1.T)@W2.T   ; out = h + ff

Sharding: 8 cores = 4 batches x 2-way query-row split.  Core c owns batch
b=c//2 and interleaved 128-row chunks g = 2*j + (c%2), j=0..7 (zig-zag, so
causal-attention work is balanced across the pair).  K/V projections over
all 2048 rows are duplicated within each pair; no collectives.

On-chip layout: activations are stored transposed [feature(part), row(free)],
which lets every matmul in the layer run without any on-chip transpose:
  - proj:    out_T[f,r]   = mm(lhsT=W_T[e,f] blk,  rhs=x_T[e,r])
  - V:       V_nat[r,v]   = mm(lhsT=x_T[e,r] blk,  rhs=W_T[e,v])
  - scores:  s_T[kr,qr]   = mm(lhsT=K_T[e,kr] blk, rhs=Q_T[e,qr])
  - softmax: exp in-place (no max-sub needed; scores are O(1)), column sums
             via ones-matmul (M=128 -> pre-broadcast), causal mask as
             additive bf16 input data
  - AV:      a_T[af,qr]   = mm(lhsT=V_nat[kr,af] blk, rhs=expT[kr,qr])
  - the softmax denominator is divided out after the O-projection, fused
    into the residual add
LN gain and the 1/sqrt(E) score scale are folded into the projection
weights on the host (exact); all biases in this problem are zero.  Q1 is
projected straight from the full-row LN1 output via a strided rhs AP (each
core's owned rows are pre-swizzled to the even 128-col blocks of every
512-chunk), so no separate owned-row LN pass is needed.

Memory: SBUF is managed as a handful of program-long pools whose tags act
as free-slot rings; successive logical tensors (k1->k2->hft, q1->attn1->
xn2->q2->attn2->xn3, the eight 2MB weight images + the w2 stream, the
xn/exp/src-chunk 1MB tiles) reuse slots with WAR deps that naturally time
each prefetch DMA right when its slot's last reader finishes.  This keeps
every large DMA at least one phase ahead of its consumer, which is what
keeps the PE from ever going idle (and from HAM-rethrottling).

Numerics: matmul operands bf16; PSUM accumulation, LN stats and softmax
sums stay fp32.  The residual stream h is bf16 in SBUF (two bf16 roundings
of an O(1) stream, well inside the 2e-2 budget); the final residual add
runs in fp32 and the output is fp32.
"""

import os
import sys

import numpy as np

for _p in ("/opt/trn_rl_repo", "/root/.axon_site/_ro/trn_rl_repo"):
    if os.path.isdir(_p) and _p not in sys.path:
        sys.path.insert(0, _p)

import ml_dtypes  # noqa: E402

import concourse.bass as bass  # noqa: E402
import concourse.tile as tile  # noqa: E402
from concourse import bacc, mybir  # noqa: E402
from concourse.bass_utils import run_bass_kernel_spmd  # noqa: E402

E = 1024
S = 2048
B = 4
F = 4096
P = 128
NE = E // P          # 8 feature blocks
NF = F // P          # 32 ff blocks
NKB = S // P         # 16 key-row blocks
RO = 1024            # owned query rows per core
NCORES = 8

F32 = mybir.dt.float32
BF16 = mybir.dt.bfloat16
F8 = mybir.dt.float8e4
BF = ml_dtypes.bfloat16
F8NP = ml_dtypes.float8_e4m3
ALU = mybir.AluOpType
ACT_F = mybir.ActivationFunctionType
DR = mybir.MatmulPerfMode.DoubleRow
WS = 64.0     # host-side fp8 weight scale (2^6, exact); undone at evacuation
IS = 1.0 / WS

NEG = -1e30

_NC_CACHE = {}
LAST_RESULTS = None  # BassKernelResults of the most recent hardware run


def _build_program():
    """Emit the single SPMD program (identical for all 8 cores)."""
    nc = bacc.Bacc(
        "TRN2",
        target_bir_lowering=False,
        debug=False,
        enable_asserts=False,
        num_devices=NCORES,
    )

    d = {}
    d["tgt_t"] = nc.dram_tensor("tgt_t", [4, NE, P, 512], BF16, kind="ExternalInput")
    d["tgto"] = nc.dram_tensor("tgto", [P, NE * RO], BF16, kind="ExternalInput")
    d["src_t"] = nc.dram_tensor("src_t", [4, NE, P, 512], F8, kind="ExternalInput")
    d["mask"] = nc.dram_tensor("mask", [2, P, 8 * 512], BF16, kind="ExternalInput")
    for w in ("wq1", "wk1", "wv1", "wq2", "wk2", "wv2"):
        d[w] = nc.dram_tensor(w, [P, NE * E], F8, kind="ExternalInput")
    for w in ("wo1", "wo2"):
        d[w] = nc.dram_tensor(w, [P, NE * E], BF16, kind="ExternalInput")
    d["w1"] = nc.dram_tensor("w1", [NF, P, NE * P], BF16, kind="ExternalInput")
    d["w2"] = nc.dram_tensor("w2", [NE, P, NF * P], BF16, kind="ExternalInput")
    d["out_t"] = nc.dram_tensor("out_t", [P, NE * RO], F32, kind="ExternalOutput")

    with tile.TileContext(nc) as tc:
        with nc.allow_low_precision(
                reason="bf16 LN stats / softmax inv are within the 2e-2 "
                       "relative-error budget (validated in sim)"):
            _emit(tc, {k: v.ap() for k, v in d.items()})

    nc.compile()
    return nc


def _emit(tc, d):
    nc = tc.nc

    # --- PSUM: one pool, 8 banks total across tags -------------------------
    ps = tc.alloc_tile_pool(name="ps", bufs=1, space="PSUM")

    def ps_tile(name, tag, bufs, shape=(P, 512)):
        return ps.tile(list(shape), F32, name=name, tag=tag, bufs=bufs)

    # --- SBUF: program-long pools; tags are free-slot rings ----------------
    def pool(name, bufs=1, side="left"):
        return tc.alloc_tile_pool(name=name, bufs=bufs, side=side)

    consts = pool("consts")
    ones_1 = consts.tile([P, P], BF16, name="ones_1", tag="ones_1")
    nc.vector.memset(ones_1[:], 1.0)
    # 1/E (= 2^-10, exact in bf16) folded into the LN stat sums
    ones_m = consts.tile([P, P], BF16, name="ones_m", tag="ones_m")
    nc.vector.memset(ones_m[:], 1.0 / E)
    eps_t = consts.tile([P, 1], F32, name="eps_t", tag="eps")
    nc.vector.memset(eps_t[:], 1e-5)

    tmp = pool("tmp", bufs=1)        # LN chain scratch, tags t0/t1 (f32)
    sq8 = pool("sq8", bufs=4)        # bf16 squares (DVE) for LN sum(x^2)
    statp = pool("statp", bufs=2)    # LN A/B stat tiles (bf16, 2 rc in flight)
    invp = pool("invp", bufs=1)      # softmax 1/sum tiles (bf16, tags i0/i1)
    st5 = pool("st5", bufs=8)        # [P,512] bf16 stream: tgt-in/mask/res
    outp = pool("outp", bufs=1)      # [P,512] f32 output staging
    warena = pool("warena", bufs=3)  # 2MB slots: 8 proj weights + w2 stream
    kvp = pool("kvp", bufs=2)        # 4MB slots: k1,v1 -> k2,v2 -> hft a/b
    qxp = pool("qxp", bufs=2)        # 2MB slots: q1,attn1 -> xn2,q2 -> attn2,xn3
    xep = pool("xep", bufs=2)        # 1MB slots: xn chunks, exp tiles, src chunks
    hpool = pool("hpool")            # residual stream h (bf16, 2MB)
    w1p = pool("w1p", bufs=3)        # 256KB slots: w1 stream

    h = hpool.tile([P, NE * RO], BF16, name="h", tag="h")

    def w_tile(nm, dt=F8):
        t = warena.tile([P, NE * E], dt, name=nm, tag="w")
        nc.sync.dma_start(t[:], d[nm][:])
        return t

    def wpair(w, i, c0, cw):
        """[P,2,cw] k-block pair (2i,2i+1) of a [P, NE*E] weight image."""
        return w[:].rearrange("p (e c) -> p e c", e=NE)[:, 2 * i:2 * i + 2,
                                                       c0:c0 + cw]

    def ln_stats(get_x, get_sq, a_tile, b_tile, prefix):
        """Per-row LN stats over one transposed 512-chunk.

        get_x(eb) -> [P,512] bf16 AP; get_sq(eb) -> [P,512] bf16 AP of x^2.
        Fills a_tile = rsqrt(var+eps), b_tile = mean * a  (bf16, broadcast
        along partitions by the 1/E-scaled ones-matmul).
        """
        mu = ps_tile(f"{prefix}sx", "sx", 2)     # mean (1/E in ones_m)
        ex2 = ps_tile(f"{prefix}sxx", "sxx", 2)  # E[x^2]
        for eb in range(NE):
            x = get_x(eb)
            sq = get_sq(eb)
            nc.tensor.matmul(mu[:], ones_m[:], x,
                             start=(eb == 0), stop=(eb == NE - 1))
            nc.tensor.matmul(ex2[:], ones_m[:], sq,
                             start=(eb == 0), stop=(eb == NE - 1))
        v = tmp.tile([P, 512], F32, name=f"{prefix}v", tag="t1")
        nc.scalar.square(v[:], mu[:])
        nc.vector.scalar_tensor_tensor(
            v[:], ex2[:], 1.0, v[:], ALU.mult, ALU.subtract)
        nc.scalar.activation(v[:], v[:], ACT_F.Sqrt, bias=eps_t[:])
        nc.vector.reciprocal_approx_fast(v[:], v[:])
        nc.scalar.copy(a_tile[:], v[:])
        nc.vector.tensor_mul(b_tile[:], mu[:], a_tile[:])

    def ln_apply(dst, src_ap, a, bv, prefix):
        """dst (bf16) = src*A - Bv (bf16 throughout for DVE fast modes)."""
        t = tmp.tile([P, 512], BF16, name=f"{prefix}ap", tag="t0")
        nc.vector.tensor_mul(t[:], src_ap, a[:])
        nc.vector.tensor_sub(dst, t[:], bv[:])

    def stat_tiles(nm):
        a = statp.tile([P, 512], BF16, name=f"{nm}A", tag="A")
        b = statp.tile([P, 512], BF16, name=f"{nm}B", tag="B")
        return a, b

    # ============== phase A: LN1 over all rows -> K1, V1, Q1 ===============
    k1 = kvp.tile([P, NE * S], BF16, name="k1", tag="kv")
    v1 = kvp.tile([P, NKB * E], BF16, name="v1", tag="kv")
    q1 = qxp.tile([P, NE * RO], BF16, name="q1", tag="qx")

    T = [None] * 4

    def load_rc(rc):
        tiles = []
        for eb in range(NE):
            t = st5.tile([P, 512], BF16, name=f"tgt{rc}_{eb}", tag="s5")
            nc.sync.dma_start(t[:], d["tgt_t"][rc, eb])
            tiles.append(t)
        T[rc] = tiles

    # DMA order: first stats chunk, then the phase-A weights
    load_rc(0)
    wk1 = w_tile("wk1")
    load_rc(1)
    wv1 = w_tile("wv1")
    wq1 = w_tile("wq1")

    def stats1(rc):
        sqs = []
        for eb in range(NE):
            sq = sq8.tile([P, 512], BF16, name=f"l1sq{rc}_{eb}", tag="sq")
            nc.gpsimd.tensor_mul(sq[:], T[rc][eb][:], T[rc][eb][:])
            sqs.append(sq)
        a, b = stat_tiles(f"l1f{rc}")
        ln_stats(lambda eb: T[rc][eb][:], lambda eb: sqs[eb][:], a, b,
                 f"l1f{rc}")
        return a, b

    xn = [None] * 4

    def apply1(rc, ab):
        a, b = ab
        x = xep.tile([P, NE * 512], F8, name=f"xn{rc}", tag="xe")
        for eb in range(NE):
            ln_apply(x[:, eb * 512:eb * 512 + 512], T[rc][eb][:], a, b,
                     f"l1a{rc}")
        xn[rc] = x

    def xnpair(x, i, c0, cw):
        return x[:].rearrange("p (e c) -> p e c", e=NE)[:, 2 * i:2 * i + 2,
                                                        c0:c0 + cw]

    def k1_proj(rc):
        for kf in range(NE):
            kp = ps_tile("kp", "mm", 2)
            for i in range(NE // 2):
                nc.tensor.matmul(
                    kp[:], wpair(wk1, i, kf * P, P), xnpair(xn[rc], i, 0, 512),
                    start=(i == 0), stop=(i == NE // 2 - 1), perf_mode=DR)
            nc.scalar.activation(
                k1[:, kf * S + rc * 512:kf * S + rc * 512 + 512], kp[:],
                ACT_F.Copy, scale=IS)

    def v1_proj(rc):
        for rb in range(4):
            for vf in range(2):
                vp = ps_tile("vp", "mm", 2)
                for i in range(NE // 2):
                    nc.tensor.matmul(
                        vp[:], xnpair(xn[rc], i, rb * P, P),
                        wpair(wv1, i, vf * 512, 512),
                        start=(i == 0), stop=(i == NE // 2 - 1), perf_mode=DR)
                o = (rc * 4 + rb) * E + vf * 512
                nc.scalar.activation(v1[:, o:o + 512], vp[:], ACT_F.Copy,
                                     scale=IS)

    def q1_proj(c):
        # own chunk c (512 cols) = the even 128-col blocks of xn[2c],
        # xn[2c+1] (the host pre-swizzles tgt so each core's owned rows land
        # at even block positions; mask/tgto follow the swizzle).
        for half in range(2):
            rc = 2 * c + half
            xv = xn[rc][:].rearrange("p (e b t c) -> p e t b c",
                                     e=NE, b=2, t=2, c=P)
            for fblk in range(NE):
                qp = ps_tile("qp", "mm", 2, shape=(P, 256))
                for eb in range(NE):
                    nc.tensor.matmul(
                        qp[:],
                        wq1[:, eb * E + fblk * P:eb * E + fblk * P + P],
                        xv[:, eb, 0],
                        start=(eb == 0), stop=(eb == NE - 1))
                o = fblk * RO + c * 512 + half * 256
                nc.scalar.activation(q1[:, o:o + 256], qp[:], ACT_F.Copy,
                                     scale=IS)

    apply1(0, stats1(0))
    apply1(1, stats1(1))
    k1_proj(0)
    v1_proj(0)
    q1_proj(0)
    load_rc(2)
    apply1(2, stats1(2))
    k1_proj(1)
    v1_proj(1)
    load_rc(3)
    apply1(3, stats1(3))
    k1_proj(2)
    v1_proj(2)
    k1_proj(3)
    v1_proj(3)
    q1_proj(1)

    # ============== attention helper =======================================
    def attention(q_sb, k_sb, v_sb, masked, prefix, attn):
        """Softmax attention; normalized output goes to attn (bf16)."""
        for t in range(2):
            ext = (8 * (t + 1)) if masked else NKB
            nhalf = (ext + 7) // 8
            ets = [xep.tile([P, 8 * 512], BF16, name=f"{prefix}et{t}_{i}",
                            tag="xe") for i in range(nhalf)]

            def et_sl(kb):
                return ets[kb // 8][:, (kb % 8) * 512:(kb % 8) * 512 + 512]

            for kb in range(ext):
                sp = ps_tile(f"{prefix}sp", "mm", 2)
                for eb in range(NE):
                    nc.tensor.matmul(
                        sp[:],
                        k_sb[:, eb * S + kb * P:eb * S + kb * P + P],
                        q_sb[:, eb * RO + t * 512:eb * RO + t * 512 + 512],
                        start=(eb == 0), stop=(eb == NE - 1))
                if masked and kb >= 8 * t:
                    mo = (kb - 8 * t) * 512
                    mt = st5.tile([P, 512], BF16, name=f"{prefix}mt{t}_{kb}",
                                  tag="s5")
                    nc.sync.dma_start(mt[:], d["mask"][t, :, mo:mo + 512])
                    nc.vector.tensor_add(sp[:], sp[:], mt[:])
                nc.scalar.activation(et_sl(kb), sp[:], ACT_F.Exp)
            # softmax denominator: ones-matmul column sums (pre-broadcast);
            # 1/sum is folded into the AV PSUM evacuation below
            sm = ps_tile(f"{prefix}sm", "sx", 2)
            for kb in range(ext):
                nc.tensor.matmul(sm[:], ones_1[:], et_sl(kb),
                                 start=(kb == 0), stop=(kb == ext - 1))
            inv = invp.tile([P, 512], F32, name=f"{prefix}inv{t}",
                            tag=f"i{t}")
            nc.vector.reciprocal_approx_fast(inv[:], sm[:])
            for af in range(NE):
                ap_ = ps_tile(f"{prefix}avp", "av", 2)
                for kb in range(ext):
                    nc.tensor.matmul(
                        ap_[:],
                        v_sb[:, kb * E + af * P:kb * E + af * P + P],
                        et_sl(kb),
                        start=(kb == 0), stop=(kb == ext - 1))
                o = af * RO + t * 512
                nc.vector.tensor_mul(attn[:, o:o + 512], ap_[:], inv[:])

    def o_proj_residual(attn, wo, res_getter, tag, after_rc=None):
        """h[of,rc] (bf16) = W_o.T @ attn + residual, rc-major."""
        for rc in range(2):
            for of in range(NE):
                op = ps_tile(f"{tag}op", "mm", 2)
                for ab in range(NE):
                    nc.tensor.matmul(
                        op[:],
                        wo[:, ab * E + of * P:ab * E + of * P + P],
                        attn[:, ab * RO + rc * 512:ab * RO + rc * 512 + 512],
                        start=(ab == 0), stop=(ab == NE - 1))
                o = of * RO + rc * 512
                nc.vector.tensor_add(h[:, o:o + 512], op[:],
                                     res_getter(of, rc))
            if after_rc is not None:
                after_rc(rc)

    def ln_sq(rc, prefix):
        """GpSimd squares of one owned 512-chunk of h (for LN sum(x^2))."""
        sqs = []
        for eb in range(NE):
            sl = h[:, eb * RO + rc * 512:eb * RO + rc * 512 + 512]
            sq = sq8.tile([P, 512], BF16, name=f"{prefix}sq{eb}", tag="sq")
            nc.gpsimd.tensor_mul(sq[:], sl, sl)
            sqs.append(sq)
        return sqs

    # ============== self-attention + O1 ====================================
    attn1 = qxp.tile([P, NE * RO], BF16, name="attn1", tag="qx")
    # prefetch: slots for these free as phase-A weights die
    wo1 = w_tile("wo1", BF16)
    wq2 = w_tile("wq2")
    wk2 = w_tile("wk2")

    attention(q1, k1, v1, True, "sa", attn1)

    sq2 = [None, None]

    def after_o1(rc):
        sq2[rc] = ln_sq(rc, f"l2p{rc}")

    def res1(of, rc):
        rt = st5.tile([P, 512], BF16, name=f"res{of}_{rc}", tag="s5")
        o = of * RO + rc * 512
        nc.sync.dma_start(rt[:], d["tgto"][:, o:o + 512])
        return rt[:]

    o_proj_residual(attn1, wo1, res1, "o1", after_rc=after_o1)

    # prefetch (slots free at O1 end / Q2 end)
    wv2 = w_tile("wv2")
    wo2 = w_tile("wo2", BF16)

    # ============== LN2 + K2/Q2/V2 =========================================
    xn2 = qxp.tile([P, NE * RO], F8, name="xn2", tag="qx")
    q2 = qxp.tile([P, NE * RO], BF16, name="q2", tag="qx")
    k2 = kvp.tile([P, NE * S], BF16, name="k2", tag="kv")
    v2 = kvp.tile([P, NKB * E], BF16, name="v2", tag="kv")

    def load_src(rc, nm):
        tiles = xep.tile([P, NE * 512], F8, name=nm, tag="xe")
        for eb in range(NE):
            nc.sync.dma_start(tiles[:, eb * 512:eb * 512 + 512],
                              d["src_t"][rc, eb])
        return tiles

    a2b2 = [stat_tiles(f"l2{rc}") for rc in range(2)]

    def stats2(rc):
        a, b = a2b2[rc]
        ln_stats(lambda eb: h[:, eb * RO + rc * 512:eb * RO + rc * 512 + 512],
                 lambda eb: sq2[rc][eb][:], a, b, f"l2s{rc}")

    def k2_proj(rc, src_rc):
        for kf in range(NE):
            kp = ps_tile("kp2", "mm", 2)
            for i in range(NE // 2):
                nc.tensor.matmul(
                    kp[:], wpair(wk2, i, kf * P, P), xnpair(src_rc, i, 0, 512),
                    start=(i == 0), stop=(i == NE // 2 - 1), perf_mode=DR)
            nc.scalar.activation(
                k2[:, kf * S + rc * 512:kf * S + rc * 512 + 512], kp[:],
                ACT_F.Copy, scale=IS)

    srcK = load_src(0, "srcK0")
    stats2(0)
    k2_proj(0, srcK)
    srcK1 = load_src(1, "srcK1")
    stats2(1)
    for rc in range(2):
        a, b = a2b2[rc]
        for eb in range(NE):
            o = eb * RO + rc * 512
            ln_apply(xn2[:, o:o + 512], h[:, o:o + 512], a, b, f"l2a{rc}")
    k2_proj(1, srcK1)
    srcK2 = load_src(2, "srcK2")
    k2_proj(2, srcK2)
    srcK3 = load_src(3, "srcK3")
    k2_proj(3, srcK3)
    # Q2 projection (owned rows)
    xn2r = xn2[:].rearrange("p (e c) -> p e c", e=NE)
    for fblk in range(NE):
        for rc in range(2):
            qp = ps_tile("q2p", "mm", 2)
            for i in range(NE // 2):
                nc.tensor.matmul(
                    qp[:], wpair(wq2, i, fblk * P, P),
                    xn2r[:, 2 * i:2 * i + 2, rc * 512:rc * 512 + 512],
                    start=(i == 0), stop=(i == NE // 2 - 1), perf_mode=DR)
            o = fblk * RO + rc * 512
            nc.scalar.activation(q2[:, o:o + 512], qp[:], ACT_F.Copy,
                                 scale=IS)
    # V2 (re-stream src chunks)
    for rc in range(4):
        src_rc = load_src(rc, f"srcV{rc}")
        for rb in range(4):
            for vf in range(2):
                vp = ps_tile("vp2", "mm", 2)
                for i in range(NE // 2):
                    nc.tensor.matmul(
                        vp[:], xnpair(src_rc, i, rb * P, P),
                        wpair(wv2, i, vf * 512, 512),
                        start=(i == 0), stop=(i == NE // 2 - 1), perf_mode=DR)
                o = (rc * 4 + rb) * E + vf * 512
                nc.scalar.activation(v2[:, o:o + 512], vp[:], ACT_F.Copy,
                                     scale=IS)

    # ============== cross-attention + O2 (in-place residual) ===============
    attn2 = qxp.tile([P, NE * RO], BF16, name="attn2", tag="qx")

    # w1 stream prefetch (fresh slots, DMAs run during CA)
    w1_tiles = {}

    def w1_tile(fb):
        if fb not in w1_tiles:
            t = w1p.tile([P, NE * P], BF16, name=f"w1t{fb}", tag="w1")
            nc.sync.dma_start(t[:], d["w1"][fb])
            w1_tiles[fb] = t
        return w1_tiles[fb]

    for fb in range(3):
        w1_tile(fb)

    attention(q2, k2, v2, False, "ca", attn2)

    sq3 = [None, None]

    def after_o2(rc):
        sq3[rc] = ln_sq(rc, f"l3p{rc}")

    o_proj_residual(attn2, wo2,
                    lambda of, rc: h[:, of * RO + rc * 512:
                                     of * RO + rc * 512 + 512],
                    "o2", after_rc=after_o2)

    # ============== LN3 + FFN + final residual =============================
    xn3 = qxp.tile([P, NE * RO], BF16, name="xn3", tag="qx")
    hft_a = kvp.tile([P, 16 * RO], BF16, name="hft_a", tag="kv")
    hft_b = kvp.tile([P, 16 * RO], BF16, name="hft_b", tag="kv")

    def hft_sl(fb, rc):
        t = hft_a if fb < 16 else hft_b
        o = (fb % 16) * RO + rc * 512
        return t[:, o:o + 512]

    def hft_pair(j, rc):
        """[P,2,512] fb-block pair (2j,2j+1) of hft."""
        t = hft_a if 2 * j < 16 else hft_b
        r = t[:].rearrange("p (f c) -> p f c", f=16)
        jj = (2 * j) % 16
        return r[:, jj:jj + 2, rc * 512:rc * 512 + 512]

    a3b3 = [stat_tiles(f"l3{rc}") for rc in range(2)]

    def apply3(rc):
        a, b = a3b3[rc]
        for eb in range(NE):
            o = eb * RO + rc * 512
            ln_apply(xn3[:, o:o + 512], h[:, o:o + 512], a, b, f"l3a{rc}")

    for rc in range(2):
        a, b = a3b3[rc]
        ln_stats(lambda eb: h[:, eb * RO + rc * 512:eb * RO + rc * 512 + 512],
                 lambda eb: sq3[rc][eb][:], a, b, f"l3s{rc}")
        apply3(rc)

    # FF1: first rc1-groups deferred so apply3(rc1) hides behind rc0 work
    ff1_order = [(0, 0), (1, 0), (0, 1), (1, 1)] + \
        [(fb, rc) for fb in range(2, NF) for rc in range(2)]
    xn3r = xn3[:].rearrange("p (e c) -> p e c", e=NE)
    for fb, rc in ff1_order:
        w1t = w1_tile(fb)
        if rc == 0 and fb + 2 < NF:
            w1_tile(fb + 2)  # keep the w1 DMA stream two tiles ahead
        hps = ps_tile("hps", "mm", 2)
        for eb in range(NE):
            nc.tensor.matmul(
                hps[:],
                w1t[:, eb * P:eb * P + P],
                xn3[:, eb * RO + rc * 512:eb * RO + rc * 512 + 512],
                start=(eb == 0), stop=(eb == NE - 1))
        nc.scalar.activation(hft_sl(fb, rc), hps[:], ACT_F.Relu)

    # FF2 + final residual in fp32 + chunked output DMA
    w2_tiles = []

    def w2_prefetch(upto):
        while len(w2_tiles) < min(upto, NE):
            j = len(w2_tiles)
            t = warena.tile([P, NF * P], BF16, name=f"w2t{j}", tag="w")
            nc.sync.dma_start(t[:], d["w2"][j])
            w2_tiles.append(t)

    w2_prefetch(2)
    for of in range(NE):
        w2_prefetch(of + 3)
        w2t = w2_tiles[of]
        for rc in range(2):
            ops = ps_tile("ops", "mm", 2)
            for fb in range(NF):
                nc.tensor.matmul(
                    ops[:],
                    w2t[:, fb * P:fb * P + P],
                    hft_sl(fb, rc),
                    start=(fb == 0), stop=(fb == NF - 1))
            o = of * RO + rc * 512
            ot = outp.tile([P, 512], F32, name=f"out{of}_{rc}", tag="ot")
            nc.vector.tensor_add(ot[:], ops[:], h[:, o:o + 512])
            nc.sync.dma_start(d["out_t"][:, o:o + 512], ot[:])

    for p_ in (w1p, hpool, xep, qxp, kvp, warena, outp, st5, invp, statp,
               sq8, tmp, consts, ps):
        p_.release()


# ---------------------------------------------------------------------------
# host side: input swizzling, weight folding, output assembly
# ---------------------------------------------------------------------------

def _swz_w(w_t):
    """[E_in, N] (already [in, out]) -> SBUF image [P, (E_in/P)*N]."""
    e_in, n = w_t.shape
    return np.ascontiguousarray(
        w_t.reshape(e_in // P, P, n).transpose(1, 0, 2).reshape(P, -1))


def _own_rows(h):
    idx = []
    for j in range(8):
        g = 2 * j + h
        idx.extend(range(g * P, (g + 1) * P))
    return np.array(idx)


# swap even/odd 128-row groups: [1,0,3,2,5,4,...]
_BLKSWAP = np.arange(NKB).reshape(-1, 2)[:, ::-1].reshape(-1)


def _chunked(x_t):
    """[E, S] -> [4, NE, P, 512] (rc-chunk major, feature-block, part)."""
    return np.ascontiguousarray(
        x_t.reshape(NE, P, 4, 512).transpose(2, 0, 1, 3))


def make_in_maps(inputs):
    f32 = np.float32
    tgt = np.asarray(inputs["tgt_embs"], f32)
    src = np.asarray(inputs["src_encs"], f32)

    g1 = np.asarray(inputs["ln1_g"], f32)
    g2 = np.asarray(inputs["ln2_g"], f32)
    g3 = np.asarray(inputs["ln3_g"], f32)
    for nm in ("sa_bq", "sa_bk", "sa_bv", "sa_bo", "ca_bq", "ca_bk", "ca_bv",
               "ca_bo", "ff_b1", "ff_b2", "ln1_b", "ln2_b", "ln3_b"):
        assert np.abs(np.asarray(inputs[nm])).max() == 0.0, \
            f"nonzero bias {nm} not supported"

    scale = f32(1.0 / np.sqrt(E))
    wq1 = np.asarray(inputs["sa_Wq"], f32) * g1[None, :] * scale
    wk1 = np.asarray(inputs["sa_Wk"], f32) * g1[None, :]
    wv1 = np.asarray(inputs["sa_Wv"], f32) * g1[None, :]
    wo1 = np.asarray(inputs["sa_Wo"], f32)
    wq2 = np.asarray(inputs["ca_Wq"], f32) * g2[None, :] * scale
    wk2 = np.asarray(inputs["ca_Wk"], f32)
    wv2 = np.asarray(inputs["ca_Wv"], f32)
    wo2 = np.asarray(inputs["ca_Wo"], f32)
    w1 = np.asarray(inputs["ff_W1"], f32) * g3[None, :]
    w2 = np.asarray(inputs["ff_W2"], f32)

    ws = np.float32(WS)
    w_sb = {
        "wq1": _swz_w((wq1.T * ws).astype(F8NP)),
        "wk1": _swz_w((wk1.T * ws).astype(F8NP)),
        "wv1": _swz_w((wv1.T * ws).astype(F8NP)),
        "wo1": _swz_w(wo1.T.astype(BF)),
        "wq2": _swz_w((wq2.T * ws).astype(F8NP)),
        "wk2": _swz_w((wk2.T * ws).astype(F8NP)),
        "wv2": _swz_w((wv2.T * ws).astype(F8NP)),
        "wo2": _swz_w(wo2.T.astype(BF)),
    }
    w1t = w1.T.astype(BF)  # [E, F]
    w1_sw = np.ascontiguousarray(
        w1t.reshape(NE, P, NF, P).transpose(2, 1, 0, 3).reshape(NF, P, NE * P))
    w2t = w2.T.astype(BF)  # [F, E]
    w2_sw = np.ascontiguousarray(
        w2t.reshape(NF, P, NE, P).transpose(2, 1, 0, 3).reshape(NE, P, NF * P))

    in_maps = []
    for c in range(NCORES):
        b, h = c // 2, c % 2
        rows = _own_rows(h)
        # perm: physical row position -> original row index (h=1 swaps each
        # even/odd 128-row group pair so owned groups land at even positions)
        if h == 1:
            perm = (_BLKSWAP[:, None] * P + np.arange(P)[None, :]).reshape(-1)
        else:
            perm = np.arange(S)
        tgt_t = _chunked(tgt[b][perm].T).astype(BF)
        tgto = _swz_w(np.ascontiguousarray(tgt[b][rows].T)).astype(BF)
        src_t = _chunked(src[b].T).astype(F8NP)
        mask = np.zeros((2, 8, P, 512), np.float32)
        for t in range(2):
            kr = perm[1024 * t:1024 * t + 1024]  # original index of each key
            qg = np.empty(512, np.int64)
            for s in range(4):
                g = 8 * t + 2 * s + h
                qg[s * P:(s + 1) * P] = g * P + np.arange(P)
            m = np.where(kr[:, None] <= qg[None, :], 0.0, NEG).astype(np.float32)
            mask[t] = m.reshape(8, P, 512)
        # kernel mask layout: [t, P, kb*512]
        mask_k = np.ascontiguousarray(mask.transpose(0, 2, 1, 3)
                                      .reshape(2, P, 8 * 512))
        in_maps.append({
            "tgt_t": tgt_t,
            "tgto": tgto,
            "src_t": src_t,
            "mask": mask_k.astype(BF),
            **w_sb,
            "w1": w1_sw,
            "w2": w2_sw,
        })
    return in_maps


def assemble_output(results):
    out = np.empty((B, S, E), np.float32)
    for c in range(NCORES):
        b, h = c // 2, c % 2
        arr = np.asarray(results[c]["out_t"])  # [P, NE*RO]
        a = arr.reshape(P, NE, 8, P).transpose(2, 3, 1, 0).reshape(8, P, E)
        for j in range(8):
            g = 2 * j + h
            out[b, g * P:(g + 1) * P, :] = a[j]
    return out


def get_nc():
    if "nc" not in _NC_CACHE:
        _NC_CACHE["nc"] = _build_program()
    return _NC_CACHE["nc"]


def _axon_reset():
    """Recover a wedged remote NeuronCore (NRT_EXEC_UNIT_UNRECOVERABLE)."""
    try:
        import ctypes
        lib = ctypes.CDLL("/opt/axon/libaxon_pjrt.so")
        lib.axon_reset.restype = ctypes.c_int64
        lib.axon_reset()
    except Exception:
        pass


def kernel(**inputs):
    global LAST_RESULTS
    in_maps = make_in_maps(inputs)
    nc = get_nc()
    last_err = None
    for attempt in range(3):
        try:
            res = run_bass_kernel_spmd(nc, in_maps, list(range(NCORES)))
            break
        except Exception as e:  # wedged device -> reset + retry
            last_err = e
            _axon_reset()
    else:
        raise last_err
    LAST_RESULTS = res
    return assemble_output(res.results)


# revision 49
# speedup vs baseline: 1.3264x; 1.1495x over previous
"""Trainium2 Bass kernel for a single-head transformer decoder layer.

Model (per batch element, S=2048, E=1024, F=4096):
    xn  = LN(tgt);  sa = causal_attn(xn)       ; h   = tgt + sa
    xn2 = LN(h);    ca = cross_attn(xn2, src)  ; h  += ca
    xn3 = LN(h);    ff = relu(xn3@W1.T)@W2.T   ; out = h + ff

Sharding: 8 cores = 4 batches x 2-way query-row split.  Core c owns batch
b=c//2 and interleaved 128-row chunks g = 2*j + (c%2), j=0..7 (zig-zag, so
causal-attention work is balanced across the pair).  K/V projections over
all 2048 rows are duplicated within each pair; no collectives.

On-chip layout: activations are stored transposed [feature(part), row(free)],
which lets every matmul in the layer run without any on-chip transpose:
  - proj:    out_T[f,r]   = mm(lhsT=W_T[e,f] blk,  rhs=x_T[e,r])
  - V:       V_nat[r,v]   = mm(lhsT=x_T[e,r] blk,  rhs=W_T[e,v])
  - scores:  s_T[kr,qr]   = mm(lhsT=K_T[e,kr] blk, rhs=Q_T[e,qr])
  - softmax: exp in-place (no max-sub needed; scores are O(1)), column sums
             via ones-matmul (M=128 -> pre-broadcast), causal mask as
             additive bf16 input data
  - AV:      a_T[af,qr]   = mm(lhsT=V_nat[kr,af] blk, rhs=expT[kr,qr])
  - the softmax denominator is divided out after the O-projection, fused
    into the residual add
LN gain and the 1/sqrt(E) score scale are folded into the projection
weights on the host (exact); all biases in this problem are zero.  Q1 is
projected straight from the full-row LN1 output via a strided rhs AP (each
core's owned rows are pre-swizzled to the even 128-col blocks of every
512-chunk), so no separate owned-row LN pass is needed.

Memory: SBUF is managed as a handful of program-long pools whose tags act
as free-slot rings; successive logical tensors (k1->k2->hft, q1->attn1->
xn2->q2->attn2->xn3, the eight 2MB weight images + the w2 stream, the
xn/exp/src-chunk 1MB tiles) reuse slots with WAR deps that naturally time
each prefetch DMA right when its slot's last reader finishes.  This keeps
every large DMA at least one phase ahead of its consumer, which is what
keeps the PE from ever going idle (and from HAM-rethrottling).

Numerics: matmul operands bf16; PSUM accumulation, LN stats and softmax
sums stay fp32.  The residual stream h is bf16 in SBUF (two bf16 roundings
of an O(1) stream, well inside the 2e-2 budget); the final residual add
runs in fp32 and the output is fp32.
"""

import os
import sys

import numpy as np

for _p in ("/opt/trn_rl_repo", "/root/.axon_site/_ro/trn_rl_repo"):
    if os.path.isdir(_p) and _p not in sys.path:
        sys.path.insert(0, _p)

import ml_dtypes  # noqa: E402

import concourse.bass as bass  # noqa: E402
import concourse.tile as tile  # noqa: E402
from concourse import bacc, mybir  # noqa: E402
from concourse.bass_utils import run_bass_kernel_spmd  # noqa: E402

E = 1024
S = 2048
B = 4
F = 4096
P = 128
NE = E // P          # 8 feature blocks
NF = F // P          # 32 ff blocks
NKB = S // P         # 16 key-row blocks
RO = 1024            # owned query rows per core
NCORES = 8

F32 = mybir.dt.float32
BF16 = mybir.dt.bfloat16
F8 = mybir.dt.float8e4
BF = ml_dtypes.bfloat16
F8NP = ml_dtypes.float8_e4m3
ALU = mybir.AluOpType
ACT_F = mybir.ActivationFunctionType
DR = mybir.MatmulPerfMode.DoubleRow
WS = 64.0     # host-side fp8 weight scale (2^6, exact); undone at evacuation
IS = 1.0 / WS

NEG = -1e30

_NC_CACHE = {}
LAST_RESULTS = None  # BassKernelResults of the most recent hardware run


def _build_program():
    """Emit the single SPMD program (identical for all 8 cores)."""
    nc = bacc.Bacc(
        "TRN2",
        target_bir_lowering=False,
        debug=False,
        enable_asserts=False,
        num_devices=NCORES,
    )

    d = {}
    d["tgt_t"] = nc.dram_tensor("tgt_t", [4, NE, P, 512], BF16, kind="ExternalInput")
    d["tgto"] = nc.dram_tensor("tgto", [P, NE * RO], BF16, kind="ExternalInput")
    d["src_t"] = nc.dram_tensor("src_t", [4, NE, P, 512], F8, kind="ExternalInput")
    d["mask"] = nc.dram_tensor("mask", [2, P, 8 * 512], BF16, kind="ExternalInput")
    for w in ("wq1", "wk1", "wv1", "wq2", "wk2", "wv2"):
        d[w] = nc.dram_tensor(w, [P, NE * E], F8, kind="ExternalInput")
    for w in ("wo1", "wo2"):
        d[w] = nc.dram_tensor(w, [P, NE * E], BF16, kind="ExternalInput")
    d["w1"] = nc.dram_tensor("w1", [NF, P, NE * P], BF16, kind="ExternalInput")
    d["w2"] = nc.dram_tensor("w2", [NE, P, NF * P], BF16, kind="ExternalInput")
    d["out_t"] = nc.dram_tensor("out_t", [P, NE * RO], F32, kind="ExternalOutput")

    with tile.TileContext(nc) as tc:
        with nc.allow_low_precision(
                reason="bf16 LN stats / softmax inv are within the 2e-2 "
                       "relative-error budget (validated in sim)"):
            _emit(tc, {k: v.ap() for k, v in d.items()})

    nc.compile()
    return nc


def _emit(tc, d):
    nc = tc.nc

    # --- PSUM: one pool, 8 banks total across tags -------------------------
    ps = tc.alloc_tile_pool(name="ps", bufs=1, space="PSUM")

    def ps_tile(name, tag, bufs, shape=(P, 512)):
        return ps.tile(list(shape), F32, name=name, tag=tag, bufs=bufs)

    # --- SBUF: program-long pools; tags are free-slot rings ----------------
    def pool(name, bufs=1, side="left"):
        return tc.alloc_tile_pool(name=name, bufs=bufs, side=side)

    consts = pool("consts")
    ones_1 = consts.tile([P, P], F8, name="ones_1", tag="ones_1")
    nc.vector.memset(ones_1[:], 1.0)
    # 1/E (= 2^-10, exact in bf16) folded into the LN stat sums
    ones_m = consts.tile([P, P], BF16, name="ones_m", tag="ones_m")
    nc.vector.memset(ones_m[:], 1.0 / E)
    eps_t = consts.tile([P, 1], F32, name="eps_t", tag="eps")
    nc.vector.memset(eps_t[:], 1e-5)

    tmp = pool("tmp", bufs=1)        # LN chain scratch, tags t0/t1 (f32)
    sq8 = pool("sq8", bufs=4)        # bf16 squares (DVE) for LN sum(x^2)
    statp = pool("statp", bufs=2)    # LN A/B stat tiles (bf16, 2 rc in flight)
    invp = pool("invp", bufs=1)      # softmax 1/sum tiles (bf16, tags i0/i1)
    st5 = pool("st5", bufs=8)        # [P,512] bf16 stream: tgt-in/mask/res
    outp = pool("outp", bufs=1)      # [P,512] f32 output staging
    warena = pool("warena", bufs=3)  # 2MB slots: 8 proj weights + w2 stream
    kvp = pool("kvp", bufs=2)        # 4MB slots: k1,v1 -> k2,v2 -> hft a/b
    qxp = pool("qxp", bufs=2)        # 2MB slots: q1,attn1 -> xn2,q2 -> attn2,xn3
    xep = pool("xep", bufs=2)        # 1MB slots: xn chunks, exp tiles, src chunks
    hpool = pool("hpool")            # residual stream h (bf16, 2MB)
    w1p = pool("w1p", bufs=3)        # 256KB slots: w1 stream

    h = hpool.tile([P, NE * RO], BF16, name="h", tag="h")

    def w_tile(nm, dt=F8):
        t = warena.tile([P, NE * E], dt, name=nm, tag="w")
        nc.sync.dma_start(t[:], d[nm][:])
        return t

    def wpair(w, i, c0, cw):
        """[P,2,cw] k-block pair (2i,2i+1) of a [P, NE*E] weight image."""
        return w[:].rearrange("p (e c) -> p e c", e=NE)[:, 2 * i:2 * i + 2,
                                                       c0:c0 + cw]

    def ln_stats(get_x, get_sq, a_tile, b_tile, prefix):
        """Per-row LN stats over one transposed 512-chunk.

        get_x(eb) -> [P,512] bf16 AP; get_sq(eb) -> [P,512] bf16 AP of x^2.
        Fills a_tile = rsqrt(var+eps), b_tile = mean * a  (bf16, broadcast
        along partitions by the 1/E-scaled ones-matmul).
        """
        mu = ps_tile(f"{prefix}sx", "sx", 2)     # mean (1/E in ones_m)
        ex2 = ps_tile(f"{prefix}sxx", "sxx", 2)  # E[x^2]
        for eb in range(NE):
            x = get_x(eb)
            sq = get_sq(eb)
            nc.tensor.matmul(mu[:], ones_m[:], x,
                             start=(eb == 0), stop=(eb == NE - 1))
            nc.tensor.matmul(ex2[:], ones_m[:], sq,
                             start=(eb == 0), stop=(eb == NE - 1))
        v = tmp.tile([P, 512], F32, name=f"{prefix}v", tag="t1")
        nc.scalar.square(v[:], mu[:])
        nc.vector.scalar_tensor_tensor(
            v[:], ex2[:], 1.0, v[:], ALU.mult, ALU.subtract)
        nc.scalar.activation(v[:], v[:], ACT_F.Sqrt, bias=eps_t[:])
        nc.vector.reciprocal_approx_fast(v[:], v[:])
        nc.scalar.copy(a_tile[:], v[:])
        nc.vector.tensor_mul(b_tile[:], mu[:], a_tile[:])

    def ln_apply(dst, src_ap, a, bv, prefix):
        """dst (bf16) = src*A - Bv (bf16 throughout for DVE fast modes)."""
        t = tmp.tile([P, 512], BF16, name=f"{prefix}ap", tag="t0")
        nc.vector.tensor_mul(t[:], src_ap, a[:])
        nc.vector.tensor_sub(dst, t[:], bv[:])

    def stat_tiles(nm):
        a = statp.tile([P, 512], BF16, name=f"{nm}A", tag="A")
        b = statp.tile([P, 512], BF16, name=f"{nm}B", tag="B")
        return a, b

    # ============== phase A: LN1 over all rows -> K1, V1, Q1 ===============
    k1 = kvp.tile([P, NE * S], F8, name="k1", tag="kv")
    v1 = kvp.tile([P, NKB * E], F8, name="v1", tag="kv")
    q1 = qxp.tile([P, NE * RO], F8, name="q1", tag="qx")

    T = [None] * 4

    def load_rc(rc):
        tiles = []
        for eb in range(NE):
            t = st5.tile([P, 512], BF16, name=f"tgt{rc}_{eb}", tag="s5")
            nc.sync.dma_start(t[:], d["tgt_t"][rc, eb])
            tiles.append(t)
        T[rc] = tiles

    # DMA order: first stats chunk, then the phase-A weights
    load_rc(0)
    wk1 = w_tile("wk1")
    load_rc(1)
    wv1 = w_tile("wv1")
    wq1 = w_tile("wq1")

    def stats1(rc):
        sqs = []
        for eb in range(NE):
            sq = sq8.tile([P, 512], BF16, name=f"l1sq{rc}_{eb}", tag="sq")
            nc.gpsimd.tensor_mul(sq[:], T[rc][eb][:], T[rc][eb][:])
            sqs.append(sq)
        a, b = stat_tiles(f"l1f{rc}")
        ln_stats(lambda eb: T[rc][eb][:], lambda eb: sqs[eb][:], a, b,
                 f"l1f{rc}")
        return a, b

    xn = [None] * 4

    def apply1(rc, ab):
        a, b = ab
        x = xep.tile([P, NE * 512], F8, name=f"xn{rc}", tag="xe")
        for eb in range(NE):
            ln_apply(x[:, eb * 512:eb * 512 + 512], T[rc][eb][:], a, b,
                     f"l1a{rc}")
        xn[rc] = x

    def xnpair(x, i, c0, cw):
        return x[:].rearrange("p (e c) -> p e c", e=NE)[:, 2 * i:2 * i + 2,
                                                        c0:c0 + cw]

    def k1_proj(rc):
        for kf in range(NE):
            kp = ps_tile("kp", "mm", 2)
            for i in range(NE // 2):
                nc.tensor.matmul(
                    kp[:], wpair(wk1, i, kf * P, P), xnpair(xn[rc], i, 0, 512),
                    start=(i == 0), stop=(i == NE // 2 - 1), perf_mode=DR)
            nc.scalar.activation(
                k1[:, kf * S + rc * 512:kf * S + rc * 512 + 512], kp[:],
                ACT_F.Copy, scale=IS)

    def v1_proj(rc):
        for rb in range(4):
            for vf in range(2):
                vp = ps_tile("vp", "mm", 2)
                for i in range(NE // 2):
                    nc.tensor.matmul(
                        vp[:], xnpair(xn[rc], i, rb * P, P),
                        wpair(wv1, i, vf * 512, 512),
                        start=(i == 0), stop=(i == NE // 2 - 1), perf_mode=DR)
                o = (rc * 4 + rb) * E + vf * 512
                nc.scalar.activation(v1[:, o:o + 512], vp[:], ACT_F.Copy,
                                     scale=IS)

    def q1_proj(c):
        # own chunk c (512 cols) = the even 128-col blocks of xn[2c],
        # xn[2c+1] (the host pre-swizzles tgt so each core's owned rows land
        # at even block positions; mask/tgto follow the swizzle).
        for half in range(2):
            rc = 2 * c + half
            xv = xn[rc][:].rearrange("p (e b t c) -> p e t b c",
                                     e=NE, b=2, t=2, c=P)
            for fblk in range(NE):
                qp = ps_tile("qp", "mm", 2, shape=(P, 256))
                for eb in range(NE):
                    nc.tensor.matmul(
                        qp[:],
                        wq1[:, eb * E + fblk * P:eb * E + fblk * P + P],
                        xv[:, eb, 0],
                        start=(eb == 0), stop=(eb == NE - 1))
                o = fblk * RO + c * 512 + half * 256
                nc.scalar.activation(q1[:, o:o + 256], qp[:], ACT_F.Copy,
                                     scale=IS)

    apply1(0, stats1(0))
    apply1(1, stats1(1))
    k1_proj(0)
    v1_proj(0)
    q1_proj(0)
    load_rc(2)
    apply1(2, stats1(2))
    k1_proj(1)
    v1_proj(1)
    load_rc(3)
    apply1(3, stats1(3))
    k1_proj(2)
    v1_proj(2)
    k1_proj(3)
    v1_proj(3)
    q1_proj(1)

    # ============== attention helper =======================================
    def attention(q_sb, k_sb, v_sb, masked, prefix, attn):
        """Softmax attention; normalized output goes to attn (bf16)."""
        for t in range(2):
            ext = (8 * (t + 1)) if masked else NKB
            nhalf = (ext + 7) // 8
            ets = [xep.tile([P, 8 * 512], F8, name=f"{prefix}et{t}_{i}",
                            tag="xe") for i in range(nhalf)]

            def et_sl(kb):
                return ets[kb // 8][:, (kb % 8) * 512:(kb % 8) * 512 + 512]

            kr = k_sb[:].rearrange("p (e c) -> p e c", e=NE)
            qr = q_sb[:].rearrange("p (e c) -> p e c", e=NE)
            for kb in range(ext):
                sp = ps_tile(f"{prefix}sp", "mm", 2)
                for i in range(NE // 2):
                    nc.tensor.matmul(
                        sp[:],
                        kr[:, 2 * i:2 * i + 2, kb * P:kb * P + P],
                        qr[:, 2 * i:2 * i + 2, t * 512:t * 512 + 512],
                        start=(i == 0), stop=(i == NE // 2 - 1), perf_mode=DR)
                if masked and kb >= 8 * t:
                    mo = (kb - 8 * t) * 512
                    mt = st5.tile([P, 512], BF16, name=f"{prefix}mt{t}_{kb}",
                                  tag="s5")
                    nc.sync.dma_start(mt[:], d["mask"][t, :, mo:mo + 512])
                    nc.vector.tensor_add(sp[:], sp[:], mt[:])
                nc.scalar.activation(et_sl(kb), sp[:], ACT_F.Exp)
            # softmax denominator: ones-matmul column sums (pre-broadcast);
            # 1/sum is folded into the AV PSUM evacuation below
            sm = ps_tile(f"{prefix}sm", "sx", 2)
            for kb in range(ext):
                nc.tensor.matmul(sm[:], ones_1[:], et_sl(kb),
                                 start=(kb == 0), stop=(kb == ext - 1))
            inv = invp.tile([P, 512], F32, name=f"{prefix}inv{t}",
                            tag=f"i{t}")
            nc.vector.reciprocal_approx_fast(inv[:], sm[:])
            vr = v_sb[:].rearrange("p (k c) -> p k c", k=NKB)
            for af in range(NE):
                ap_ = ps_tile(f"{prefix}avp", "av", 2)
                for jj in range(ext // 2):
                    etr = ets[(2 * jj) // 8][:].rearrange(
                        "p (k c) -> p k c", k=8)
                    kk = (2 * jj) % 8
                    nc.tensor.matmul(
                        ap_[:],
                        vr[:, 2 * jj:2 * jj + 2, af * P:af * P + P],
                        etr[:, kk:kk + 2, :],
                        start=(jj == 0), stop=(jj == ext // 2 - 1),
                        perf_mode=DR)
                o = af * RO + t * 512
                nc.vector.tensor_mul(attn[:, o:o + 512], ap_[:], inv[:])

    def o_proj_residual(attn, wo, res_getter, tag, after_rc=None):
        """h[of,rc] (bf16) = W_o.T @ attn + residual, rc-major."""
        for rc in range(2):
            for of in range(NE):
                op = ps_tile(f"{tag}op", "mm", 2)
                for ab in range(NE):
                    nc.tensor.matmul(
                        op[:],
                        wo[:, ab * E + of * P:ab * E + of * P + P],
                        attn[:, ab * RO + rc * 512:ab * RO + rc * 512 + 512],
                        start=(ab == 0), stop=(ab == NE - 1))
                o = of * RO + rc * 512
                nc.vector.tensor_add(h[:, o:o + 512], op[:],
                                     res_getter(of, rc))
            if after_rc is not None:
                after_rc(rc)

    def ln_sq(rc, prefix):
        """GpSimd squares of one owned 512-chunk of h (for LN sum(x^2))."""
        sqs = []
        for eb in range(NE):
            sl = h[:, eb * RO + rc * 512:eb * RO + rc * 512 + 512]
            sq = sq8.tile([P, 512], BF16, name=f"{prefix}sq{eb}", tag="sq")
            nc.gpsimd.tensor_mul(sq[:], sl, sl)
            sqs.append(sq)
        return sqs

    # ============== self-attention + O1 ====================================
    attn1 = qxp.tile([P, NE * RO], BF16, name="attn1", tag="qx")
    # prefetch: slots for these free as phase-A weights die
    wo1 = w_tile("wo1", BF16)
    wq2 = w_tile("wq2")
    wk2 = w_tile("wk2")

    attention(q1, k1, v1, True, "sa", attn1)

    sq2 = [None, None]

    def after_o1(rc):
        sq2[rc] = ln_sq(rc, f"l2p{rc}")

    def res1(of, rc):
        rt = st5.tile([P, 512], BF16, name=f"res{of}_{rc}", tag="s5")
        o = of * RO + rc * 512
        nc.sync.dma_start(rt[:], d["tgto"][:, o:o + 512])
        return rt[:]

    o_proj_residual(attn1, wo1, res1, "o1", after_rc=after_o1)

    # prefetch (slots free at O1 end / Q2 end)
    wv2 = w_tile("wv2")
    wo2 = w_tile("wo2", BF16)

    # ============== LN2 + K2/Q2/V2 =========================================
    xn2 = qxp.tile([P, NE * RO], F8, name="xn2", tag="qx")
    q2 = qxp.tile([P, NE * RO], F8, name="q2", tag="qx")
    k2 = kvp.tile([P, NE * S], F8, name="k2", tag="kv")
    v2 = kvp.tile([P, NKB * E], F8, name="v2", tag="kv")

    def load_src(rc, nm):
        tiles = xep.tile([P, NE * 512], F8, name=nm, tag="xe")
        for eb in range(NE):
            nc.sync.dma_start(tiles[:, eb * 512:eb * 512 + 512],
                              d["src_t"][rc, eb])
        return tiles

    a2b2 = [stat_tiles(f"l2{rc}") for rc in range(2)]

    def stats2(rc):
        a, b = a2b2[rc]
        ln_stats(lambda eb: h[:, eb * RO + rc * 512:eb * RO + rc * 512 + 512],
                 lambda eb: sq2[rc][eb][:], a, b, f"l2s{rc}")

    def k2_proj(rc, src_rc):
        for kf in range(NE):
            kp = ps_tile("kp2", "mm", 2)
            for i in range(NE // 2):
                nc.tensor.matmul(
                    kp[:], wpair(wk2, i, kf * P, P), xnpair(src_rc, i, 0, 512),
                    start=(i == 0), stop=(i == NE // 2 - 1), perf_mode=DR)
            nc.scalar.activation(
                k2[:, kf * S + rc * 512:kf * S + rc * 512 + 512], kp[:],
                ACT_F.Copy, scale=IS)

    srcK = load_src(0, "srcK0")
    stats2(0)
    k2_proj(0, srcK)
    srcK1 = load_src(1, "srcK1")
    stats2(1)
    for rc in range(2):
        a, b = a2b2[rc]
        for eb in range(NE):
            o = eb * RO + rc * 512
            ln_apply(xn2[:, o:o + 512], h[:, o:o + 512], a, b, f"l2a{rc}")
    k2_proj(1, srcK1)
    srcK2 = load_src(2, "srcK2")
    k2_proj(2, srcK2)
    srcK3 = load_src(3, "srcK3")
    k2_proj(3, srcK3)
    # Q2 projection (owned rows)
    xn2r = xn2[:].rearrange("p (e c) -> p e c", e=NE)
    for fblk in range(NE):
        for rc in range(2):
            qp = ps_tile("q2p", "mm", 2)
            for i in range(NE // 2):
                nc.tensor.matmul(
                    qp[:], wpair(wq2, i, fblk * P, P),
                    xn2r[:, 2 * i:2 * i + 2, rc * 512:rc * 512 + 512],
                    start=(i == 0), stop=(i == NE // 2 - 1), perf_mode=DR)
            o = fblk * RO + rc * 512
            nc.scalar.activation(q2[:, o:o + 512], qp[:], ACT_F.Copy,
                                 scale=IS)
    # V2 (re-stream src chunks)
    for rc in range(4):
        src_rc = load_src(rc, f"srcV{rc}")
        for rb in range(4):
            for vf in range(2):
                vp = ps_tile("vp2", "mm", 2)
                for i in range(NE // 2):
                    nc.tensor.matmul(
                        vp[:], xnpair(src_rc, i, rb * P, P),
                        wpair(wv2, i, vf * 512, 512),
                        start=(i == 0), stop=(i == NE // 2 - 1), perf_mode=DR)
                o = (rc * 4 + rb) * E + vf * 512
                nc.scalar.activation(v2[:, o:o + 512], vp[:], ACT_F.Copy,
                                     scale=IS)

    # ============== cross-attention + O2 (in-place residual) ===============
    attn2 = qxp.tile([P, NE * RO], BF16, name="attn2", tag="qx")

    # w1 stream prefetch (fresh slots, DMAs run during CA)
    w1_tiles = {}

    def w1_tile(fb):
        if fb not in w1_tiles:
            t = w1p.tile([P, NE * P], BF16, name=f"w1t{fb}", tag="w1")
            nc.sync.dma_start(t[:], d["w1"][fb])
            w1_tiles[fb] = t
        return w1_tiles[fb]

    for fb in range(3):
        w1_tile(fb)

    attention(q2, k2, v2, False, "ca", attn2)

    sq3 = [None, None]

    def after_o2(rc):
        sq3[rc] = ln_sq(rc, f"l3p{rc}")

    o_proj_residual(attn2, wo2,
                    lambda of, rc: h[:, of * RO + rc * 512:
                                     of * RO + rc * 512 + 512],
                    "o2", after_rc=after_o2)

    # ============== LN3 + FFN + final residual =============================
    xn3 = qxp.tile([P, NE * RO], BF16, name="xn3", tag="qx")
    hft_a = kvp.tile([P, 16 * RO], BF16, name="hft_a", tag="kv")
    hft_b = kvp.tile([P, 16 * RO], BF16, name="hft_b", tag="kv")

    def hft_sl(fb, rc):
        t = hft_a if fb < 16 else hft_b
        o = (fb % 16) * RO + rc * 512
        return t[:, o:o + 512]

    def hft_pair(j, rc):
        """[P,2,512] fb-block pair (2j,2j+1) of hft."""
        t = hft_a if 2 * j < 16 else hft_b
        r = t[:].rearrange("p (f c) -> p f c", f=16)
        jj = (2 * j) % 16
        return r[:, jj:jj + 2, rc * 512:rc * 512 + 512]

    a3b3 = [stat_tiles(f"l3{rc}") for rc in range(2)]

    def apply3(rc):
        a, b = a3b3[rc]
        for eb in range(NE):
            o = eb * RO + rc * 512
            ln_apply(xn3[:, o:o + 512], h[:, o:o + 512], a, b, f"l3a{rc}")

    for rc in range(2):
        a, b = a3b3[rc]
        ln_stats(lambda eb: h[:, eb * RO + rc * 512:eb * RO + rc * 512 + 512],
                 lambda eb: sq3[rc][eb][:], a, b, f"l3s{rc}")
        apply3(rc)

    # FF1: first rc1-groups deferred so apply3(rc1) hides behind rc0 work
    ff1_order = [(0, 0), (1, 0), (0, 1), (1, 1)] + \
        [(fb, rc) for fb in range(2, NF) for rc in range(2)]
    xn3r = xn3[:].rearrange("p (e c) -> p e c", e=NE)
    for fb, rc in ff1_order:
        w1t = w1_tile(fb)
        if rc == 0 and fb + 2 < NF:
            w1_tile(fb + 2)  # keep the w1 DMA stream two tiles ahead
        hps = ps_tile("hps", "mm", 2)
        for eb in range(NE):
            nc.tensor.matmul(
                hps[:],
                w1t[:, eb * P:eb * P + P],
                xn3[:, eb * RO + rc * 512:eb * RO + rc * 512 + 512],
                start=(eb == 0), stop=(eb == NE - 1))
        nc.scalar.activation(hft_sl(fb, rc), hps[:], ACT_F.Relu)

    # FF2 + final residual in fp32 + chunked output DMA
    w2_tiles = []

    def w2_prefetch(upto):
        while len(w2_tiles) < min(upto, NE):
            j = len(w2_tiles)
            t = warena.tile([P, NF * P], BF16, name=f"w2t{j}", tag="w")
            nc.sync.dma_start(t[:], d["w2"][j])
            w2_tiles.append(t)

    w2_prefetch(2)
    for of in range(NE):
        w2_prefetch(of + 3)
        w2t = w2_tiles[of]
        for rc in range(2):
            ops = ps_tile("ops", "mm", 2)
            for fb in range(NF):
                nc.tensor.matmul(
                    ops[:],
                    w2t[:, fb * P:fb * P + P],
                    hft_sl(fb, rc),
                    start=(fb == 0), stop=(fb == NF - 1))
            o = of * RO + rc * 512
            ot = outp.tile([P, 512], F32, name=f"out{of}_{rc}", tag="ot")
            nc.vector.tensor_add(ot[:], ops[:], h[:, o:o + 512])
            nc.sync.dma_start(d["out_t"][:, o:o + 512], ot[:])

    for p_ in (w1p, hpool, xep, qxp, kvp, warena, outp, st5, invp, statp,
               sq8, tmp, consts, ps):
        p_.release()


# ---------------------------------------------------------------------------
# host side: input swizzling, weight folding, output assembly
# ---------------------------------------------------------------------------

def _swz_w(w_t):
    """[E_in, N] (already [in, out]) -> SBUF image [P, (E_in/P)*N]."""
    e_in, n = w_t.shape
    return np.ascontiguousarray(
        w_t.reshape(e_in // P, P, n).transpose(1, 0, 2).reshape(P, -1))


def _own_rows(h):
    idx = []
    for j in range(8):
        g = 2 * j + h
        idx.extend(range(g * P, (g + 1) * P))
    return np.array(idx)


# swap even/odd 128-row groups: [1,0,3,2,5,4,...]
_BLKSWAP = np.arange(NKB).reshape(-1, 2)[:, ::-1].reshape(-1)


def _chunked(x_t):
    """[E, S] -> [4, NE, P, 512] (rc-chunk major, feature-block, part)."""
    return np.ascontiguousarray(
        x_t.reshape(NE, P, 4, 512).transpose(2, 0, 1, 3))


def make_in_maps(inputs):
    f32 = np.float32
    tgt = np.asarray(inputs["tgt_embs"], f32)
    src = np.asarray(inputs["src_encs"], f32)

    g1 = np.asarray(inputs["ln1_g"], f32)
    g2 = np.asarray(inputs["ln2_g"], f32)
    g3 = np.asarray(inputs["ln3_g"], f32)
    for nm in ("sa_bq", "sa_bk", "sa_bv", "sa_bo", "ca_bq", "ca_bk", "ca_bv",
               "ca_bo", "ff_b1", "ff_b2", "ln1_b", "ln2_b", "ln3_b"):
        assert np.abs(np.asarray(inputs[nm])).max() == 0.0, \
            f"nonzero bias {nm} not supported"

    scale = f32(1.0 / np.sqrt(E))
    wq1 = np.asarray(inputs["sa_Wq"], f32) * g1[None, :] * scale
    wk1 = np.asarray(inputs["sa_Wk"], f32) * g1[None, :]
    wv1 = np.asarray(inputs["sa_Wv"], f32) * g1[None, :]
    wo1 = np.asarray(inputs["sa_Wo"], f32)
    wq2 = np.asarray(inputs["ca_Wq"], f32) * g2[None, :] * scale
    wk2 = np.asarray(inputs["ca_Wk"], f32)
    wv2 = np.asarray(inputs["ca_Wv"], f32)
    wo2 = np.asarray(inputs["ca_Wo"], f32)
    w1 = np.asarray(inputs["ff_W1"], f32) * g3[None, :]
    w2 = np.asarray(inputs["ff_W2"], f32)

    ws = np.float32(WS)
    w_sb = {
        "wq1": _swz_w((wq1.T * ws).astype(F8NP)),
        "wk1": _swz_w((wk1.T * ws).astype(F8NP)),
        "wv1": _swz_w((wv1.T * ws).astype(F8NP)),
        "wo1": _swz_w(wo1.T.astype(BF)),
        "wq2": _swz_w((wq2.T * ws).astype(F8NP)),
        "wk2": _swz_w((wk2.T * ws).astype(F8NP)),
        "wv2": _swz_w((wv2.T * ws).astype(F8NP)),
        "wo2": _swz_w(wo2.T.astype(BF)),
    }
    w1t = w1.T.astype(BF)  # [E, F]
    w1_sw = np.ascontiguousarray(
        w1t.reshape(NE, P, NF, P).transpose(2, 1, 0, 3).reshape(NF, P, NE * P))
    w2t = w2.T.astype(BF)  # [F, E]
    w2_sw = np.ascontiguousarray(
        w2t.reshape(NF, P, NE, P).transpose(2, 1, 0, 3).reshape(NE, P, NF * P))

    in_maps = []
    for c in range(NCORES):
        b, h = c // 2, c % 2
        rows = _own_rows(h)
        # perm: physical row position -> original row index (h=1 swaps each
        # even/odd 128-row group pair so owned groups land at even positions)
        if h == 1:
            perm = (_BLKSWAP[:, None] * P + np.arange(P)[None, :]).reshape(-1)
        else:
            perm = np.arange(S)
        tgt_t = _chunked(tgt[b][perm].T).astype(BF)
        tgto = _swz_w(np.ascontiguousarray(tgt[b][rows].T)).astype(BF)
        src_t = _chunked(src[b].T).astype(F8NP)
        mask = np.zeros((2, 8, P, 512), np.float32)
        for t in range(2):
            kr = perm[1024 * t:1024 * t + 1024]  # original index of each key
            qg = np.empty(512, np.int64)
            for s in range(4):
                g = 8 * t + 2 * s + h
                qg[s * P:(s + 1) * P] = g * P + np.arange(P)
            m = np.where(kr[:, None] <= qg[None, :], 0.0, NEG).astype(np.float32)
            mask[t] = m.reshape(8, P, 512)
        # kernel mask layout: [t, P, kb*512]
        mask_k = np.ascontiguousarray(mask.transpose(0, 2, 1, 3)
                                      .reshape(2, P, 8 * 512))
        in_maps.append({
            "tgt_t": tgt_t,
            "tgto": tgto,
            "src_t": src_t,
            "mask": mask_k.astype(BF),
            **w_sb,
            "w1": w1_sw,
            "w2": w2_sw,
        })
    return in_maps


def assemble_output(results):
    out = np.empty((B, S, E), np.float32)
    for c in range(NCORES):
        b, h = c // 2, c % 2
        arr = np.asarray(results[c]["out_t"])  # [P, NE*RO]
        a = arr.reshape(P, NE, 8, P).transpose(2, 3, 1, 0).reshape(8, P, E)
        for j in range(8):
            g = 2 * j + h
            out[b, g * P:(g + 1) * P, :] = a[j]
    return out


def get_nc():
    if "nc" not in _NC_CACHE:
        _NC_CACHE["nc"] = _build_program()
    return _NC_CACHE["nc"]


def _axon_reset():
    """Recover a wedged remote NeuronCore (NRT_EXEC_UNIT_UNRECOVERABLE)."""
    try:
        import ctypes
        lib = ctypes.CDLL("/opt/axon/libaxon_pjrt.so")
        lib.axon_reset.restype = ctypes.c_int64
        lib.axon_reset()
    except Exception:
        pass


def kernel(**inputs):
    global LAST_RESULTS
    in_maps = make_in_maps(inputs)
    nc = get_nc()
    last_err = None
    for attempt in range(3):
        try:
            res = run_bass_kernel_spmd(nc, in_maps, list(range(NCORES)))
            break
        except Exception as e:  # wedged device -> reset + retry
            last_err = e
            _axon_reset()
    else:
        raise last_err
    LAST_RESULTS = res
    return assemble_output(res.results)


# revision 50
# speedup vs baseline: 1.3627x; 1.0273x over previous
"""Trainium2 Bass kernel for a single-head transformer decoder layer.

Model (per batch element, S=2048, E=1024, F=4096):
    xn  = LN(tgt);  sa = causal_attn(xn)       ; h   = tgt + sa
    xn2 = LN(h);    ca = cross_attn(xn2, src)  ; h  += ca
    xn3 = LN(h);    ff = relu(xn3@W1.T)@W2.T   ; out = h + ff

Sharding: 8 cores = 4 batches x 2-way query-row split.  Core c owns batch
b=c//2 and interleaved 128-row chunks g = 2*j + (c%2), j=0..7 (zig-zag, so
causal-attention work is balanced across the pair).  K/V projections over
all 2048 rows are duplicated within each pair; no collectives.

On-chip layout: activations are stored transposed [feature(part), row(free)],
which lets every matmul in the layer run without any on-chip transpose:
  - proj:    out_T[f,r]   = mm(lhsT=W_T[e,f] blk,  rhs=x_T[e,r])
  - V:       V_nat[r,v]   = mm(lhsT=x_T[e,r] blk,  rhs=W_T[e,v])
  - scores:  s_T[kr,qr]   = mm(lhsT=K_T[e,kr] blk, rhs=Q_T[e,qr])
  - softmax: exp in-place (no max-sub needed; scores are O(1)), column sums
             via ones-matmul (M=128 -> pre-broadcast), causal mask as
             additive bf16 input data
  - AV:      a_T[af,qr]   = mm(lhsT=V_nat[kr,af] blk, rhs=expT[kr,qr])
  - the softmax denominator is divided out after the O-projection, fused
    into the residual add
LN gain and the 1/sqrt(E) score scale are folded into the projection
weights on the host (exact); all biases in this problem are zero.  Q1 is
projected straight from the full-row LN1 output via a strided rhs AP (each
core's owned rows are pre-swizzled to the even 128-col blocks of every
512-chunk), so no separate owned-row LN pass is needed.

Memory: SBUF is managed as a handful of program-long pools whose tags act
as free-slot rings; successive logical tensors (k1->k2->hft, q1->attn1->
xn2->q2->attn2->xn3, the eight 2MB weight images + the w2 stream, the
xn/exp/src-chunk 1MB tiles) reuse slots with WAR deps that naturally time
each prefetch DMA right when its slot's last reader finishes.  This keeps
every large DMA at least one phase ahead of its consumer, which is what
keeps the PE from ever going idle (and from HAM-rethrottling).

Numerics: matmul operands bf16; PSUM accumulation, LN stats and softmax
sums stay fp32.  The residual stream h is bf16 in SBUF (two bf16 roundings
of an O(1) stream, well inside the 2e-2 budget); the final residual add
runs in fp32 and the output is fp32.
"""

import os
import sys

import numpy as np

for _p in ("/opt/trn_rl_repo", "/root/.axon_site/_ro/trn_rl_repo"):
    if os.path.isdir(_p) and _p not in sys.path:
        sys.path.insert(0, _p)

import ml_dtypes  # noqa: E402

import concourse.bass as bass  # noqa: E402
import concourse.tile as tile  # noqa: E402
from concourse import bacc, mybir  # noqa: E402
from concourse.bass_utils import run_bass_kernel_spmd  # noqa: E402

E = 1024
S = 2048
B = 4
F = 4096
P = 128
NE = E // P          # 8 feature blocks
NF = F // P          # 32 ff blocks
NKB = S // P         # 16 key-row blocks
RO = 1024            # owned query rows per core
NCORES = 8

F32 = mybir.dt.float32
BF16 = mybir.dt.bfloat16
F8 = mybir.dt.float8e4
BF = ml_dtypes.bfloat16
F8NP = ml_dtypes.float8_e4m3
ALU = mybir.AluOpType
ACT_F = mybir.ActivationFunctionType
DR = mybir.MatmulPerfMode.DoubleRow
WS = 64.0     # host-side fp8 weight scale (2^6, exact); undone at evacuation
IS = 1.0 / WS

NEG = -1e30

_NC_CACHE = {}
LAST_RESULTS = None  # BassKernelResults of the most recent hardware run


def _build_program():
    """Emit the single SPMD program (identical for all 8 cores)."""
    nc = bacc.Bacc(
        "TRN2",
        target_bir_lowering=False,
        debug=False,
        enable_asserts=False,
        num_devices=NCORES,
    )

    d = {}
    d["tgt_t"] = nc.dram_tensor("tgt_t", [4, NE, P, 512], BF16, kind="ExternalInput")
    d["tgto"] = nc.dram_tensor("tgto", [P, NE * RO], BF16, kind="ExternalInput")
    d["src_t"] = nc.dram_tensor("src_t", [4, NE, P, 512], F8, kind="ExternalInput")
    d["mask"] = nc.dram_tensor("mask", [2, P, 8 * 512], BF16, kind="ExternalInput")
    for w in ("wq1", "wk1", "wv1", "wq2", "wk2", "wv2"):
        d[w] = nc.dram_tensor(w, [P, NE * E], F8, kind="ExternalInput")
    for w in ("wo1", "wo2"):
        d[w] = nc.dram_tensor(w, [P, NE * E], BF16, kind="ExternalInput")
    d["w1"] = nc.dram_tensor("w1", [NF, P, NE * P], BF16, kind="ExternalInput")
    d["w2"] = nc.dram_tensor("w2", [NE, P, NF * P], BF16, kind="ExternalInput")
    d["out_t"] = nc.dram_tensor("out_t", [P, NE * RO], F32, kind="ExternalOutput")

    with tile.TileContext(nc) as tc:
        with nc.allow_low_precision(
                reason="bf16 LN stats / softmax inv are within the 2e-2 "
                       "relative-error budget (validated in sim)"):
            _emit(tc, {k: v.ap() for k, v in d.items()})

    nc.compile()
    return nc


def _emit(tc, d):
    nc = tc.nc

    # --- PSUM: one pool, 8 banks total across tags -------------------------
    ps = tc.alloc_tile_pool(name="ps", bufs=1, space="PSUM")

    def ps_tile(name, tag, bufs, shape=(P, 512)):
        return ps.tile(list(shape), F32, name=name, tag=tag, bufs=bufs)

    # --- SBUF: program-long pools; tags are free-slot rings ----------------
    def pool(name, bufs=1, side="left"):
        return tc.alloc_tile_pool(name=name, bufs=bufs, side=side)

    consts = pool("consts")
    ones_1 = consts.tile([P, P], F8, name="ones_1", tag="ones_1")
    nc.vector.memset(ones_1[:], 1.0)
    # 1/E (= 2^-10, exact in bf16) folded into the LN stat sums
    ones_m = consts.tile([P, P], BF16, name="ones_m", tag="ones_m")
    nc.vector.memset(ones_m[:], 1.0 / E)
    eps_t = consts.tile([P, 1], F32, name="eps_t", tag="eps")
    nc.vector.memset(eps_t[:], 1e-5)

    tmp = pool("tmp", bufs=1)        # LN chain scratch, tags t0/t1 (f32)
    sq8 = pool("sq8", bufs=6)        # bf16 squares (DVE) for LN sum(x^2)
    statp = pool("statp", bufs=2)    # LN A/B stat tiles (bf16, 2 rc in flight)
    invp = pool("invp", bufs=1)      # softmax 1/sum tiles (bf16, tags i0/i1)
    st5 = pool("st5", bufs=12)        # [P,512] bf16 stream: tgt-in/mask/res
    outp = pool("outp", bufs=1)      # [P,512] f32 output staging
    warena = pool("warena", bufs=3)  # 2MB slots: 8 proj weights + w2 stream
    kvp = pool("kvp", bufs=2)        # 4MB slots: k1,v1 -> k2,v2 -> hft a/b
    qxp = pool("qxp", bufs=2)        # 2MB slots: q1,attn1 -> xn2,q2 -> attn2,xn3
    xep = pool("xep", bufs=2)        # 1MB slots: xn chunks, exp tiles, src chunks
    hpool = pool("hpool")            # residual stream h (bf16, 2MB)
    w1p = pool("w1p", bufs=3)        # 256KB slots: w1 stream

    h = hpool.tile([P, NE * RO], BF16, name="h", tag="h")

    def w_tile(nm, dt=F8):
        t = warena.tile([P, NE * E], dt, name=nm, tag="w")
        nc.sync.dma_start(t[:], d[nm][:])
        return t

    def wpair(w, i, c0, cw):
        """[P,2,cw] k-block pair (2i,2i+1) of a [P, NE*E] weight image."""
        return w[:].rearrange("p (e c) -> p e c", e=NE)[:, 2 * i:2 * i + 2,
                                                       c0:c0 + cw]

    def ln_stats(get_x, get_sq, a_tile, b_tile, prefix):
        """Per-row LN stats over one transposed 512-chunk.

        get_x(eb) -> [P,512] bf16 AP; get_sq(eb) -> [P,512] bf16 AP of x^2.
        Fills a_tile = rsqrt(var+eps), b_tile = mean * a  (bf16, broadcast
        along partitions by the 1/E-scaled ones-matmul).
        """
        mu = ps_tile(f"{prefix}sx", "sx", 2)     # mean (1/E in ones_m)
        ex2 = ps_tile(f"{prefix}sxx", "sxx", 1)  # E[x^2]
        for eb in range(NE):
            x = get_x(eb)
            sq = get_sq(eb)
            nc.tensor.matmul(mu[:], ones_m[:], x,
                             start=(eb == 0), stop=(eb == NE - 1))
            nc.tensor.matmul(ex2[:], ones_m[:], sq,
                             start=(eb == 0), stop=(eb == NE - 1))
        v = tmp.tile([P, 512], F32, name=f"{prefix}v", tag="t1")
        nc.scalar.square(v[:], mu[:])
        nc.vector.scalar_tensor_tensor(
            v[:], ex2[:], 1.0, v[:], ALU.mult, ALU.subtract)
        nc.scalar.activation(v[:], v[:], ACT_F.Sqrt, bias=eps_t[:])
        nc.vector.reciprocal_approx_fast(v[:], v[:])
        nc.scalar.copy(a_tile[:], v[:])
        nc.vector.tensor_mul(b_tile[:], mu[:], a_tile[:])

    def ln_apply(dst, src_ap, a, bv, prefix):
        """dst (bf16) = src*A - Bv (bf16 throughout for DVE fast modes)."""
        t = tmp.tile([P, 512], BF16, name=f"{prefix}ap", tag="t0")
        nc.vector.tensor_mul(t[:], src_ap, a[:])
        nc.vector.tensor_sub(dst, t[:], bv[:])

    def stat_tiles(nm):
        a = statp.tile([P, 512], BF16, name=f"{nm}A", tag="A")
        b = statp.tile([P, 512], BF16, name=f"{nm}B", tag="B")
        return a, b

    # ============== phase A: LN1 over all rows -> K1, V1, Q1 ===============
    k1 = kvp.tile([P, NE * S], F8, name="k1", tag="kv")
    v1 = kvp.tile([P, NKB * E], F8, name="v1", tag="kv")
    q1 = qxp.tile([P, NE * RO], F8, name="q1", tag="qx")

    T = [None] * 4

    def load_rc(rc):
        tiles = []
        for eb in range(NE):
            t = st5.tile([P, 512], BF16, name=f"tgt{rc}_{eb}", tag="s5")
            nc.sync.dma_start(t[:], d["tgt_t"][rc, eb])
            tiles.append(t)
        T[rc] = tiles

    # DMA order: first stats chunk, then the phase-A weights
    load_rc(0)
    wk1 = w_tile("wk1")
    load_rc(1)
    wv1 = w_tile("wv1")
    wq1 = w_tile("wq1")

    def stats1(rc):
        sqs = []
        for eb in range(NE):
            sq = sq8.tile([P, 512], BF16, name=f"l1sq{rc}_{eb}", tag="sq")
            nc.gpsimd.tensor_mul(sq[:], T[rc][eb][:], T[rc][eb][:])
            sqs.append(sq)
        a, b = stat_tiles(f"l1f{rc}")
        ln_stats(lambda eb: T[rc][eb][:], lambda eb: sqs[eb][:], a, b,
                 f"l1f{rc}")
        return a, b

    xn = [None] * 4

    def apply1(rc, ab):
        a, b = ab
        x = xep.tile([P, NE * 512], F8, name=f"xn{rc}", tag="xe")
        for eb in range(NE):
            ln_apply(x[:, eb * 512:eb * 512 + 512], T[rc][eb][:], a, b,
                     f"l1a{rc}")
        xn[rc] = x

    def xnpair(x, i, c0, cw):
        return x[:].rearrange("p (e c) -> p e c", e=NE)[:, 2 * i:2 * i + 2,
                                                        c0:c0 + cw]

    def k1_proj(rc):
        for kf in range(NE):
            kp = ps_tile("kp", "mm", 3)
            for i in range(NE // 2):
                nc.tensor.matmul(
                    kp[:], wpair(wk1, i, kf * P, P), xnpair(xn[rc], i, 0, 512),
                    start=(i == 0), stop=(i == NE // 2 - 1), perf_mode=DR)
            nc.scalar.activation(
                k1[:, kf * S + rc * 512:kf * S + rc * 512 + 512], kp[:],
                ACT_F.Copy, scale=IS)

    def v1_proj(rc):
        for rb in range(4):
            for vf in range(2):
                vp = ps_tile("vp", "mm", 3)
                for i in range(NE // 2):
                    nc.tensor.matmul(
                        vp[:], xnpair(xn[rc], i, rb * P, P),
                        wpair(wv1, i, vf * 512, 512),
                        start=(i == 0), stop=(i == NE // 2 - 1), perf_mode=DR)
                o = (rc * 4 + rb) * E + vf * 512
                nc.scalar.activation(v1[:, o:o + 512], vp[:], ACT_F.Copy,
                                     scale=IS)

    def q1_proj(c):
        # own chunk c (512 cols) = the even 128-col blocks of xn[2c],
        # xn[2c+1] (the host pre-swizzles tgt so each core's owned rows land
        # at even block positions; mask/tgto follow the swizzle).
        for half in range(2):
            rc = 2 * c + half
            xv = xn[rc][:].rearrange("p (e b t c) -> p e t b c",
                                     e=NE, b=2, t=2, c=P)
            for fblk in range(NE):
                qp = ps_tile("qp", "mm", 3, shape=(P, 256))
                for eb in range(NE):
                    nc.tensor.matmul(
                        qp[:],
                        wq1[:, eb * E + fblk * P:eb * E + fblk * P + P],
                        xv[:, eb, 0],
                        start=(eb == 0), stop=(eb == NE - 1))
                o = fblk * RO + c * 512 + half * 256
                nc.scalar.activation(q1[:, o:o + 256], qp[:], ACT_F.Copy,
                                     scale=IS)

    apply1(0, stats1(0))
    apply1(1, stats1(1))
    k1_proj(0)
    v1_proj(0)
    q1_proj(0)
    load_rc(2)
    apply1(2, stats1(2))
    k1_proj(1)
    v1_proj(1)
    load_rc(3)
    apply1(3, stats1(3))
    k1_proj(2)
    v1_proj(2)
    k1_proj(3)
    v1_proj(3)
    q1_proj(1)

    # ============== attention helper =======================================
    def attention(q_sb, k_sb, v_sb, masked, prefix, attn):
        """Softmax attention; normalized output goes to attn (bf16)."""
        for t in range(2):
            ext = (8 * (t + 1)) if masked else NKB
            nhalf = (ext + 7) // 8
            ets = [xep.tile([P, 8 * 512], F8, name=f"{prefix}et{t}_{i}",
                            tag="xe") for i in range(nhalf)]

            def et_sl(kb):
                return ets[kb // 8][:, (kb % 8) * 512:(kb % 8) * 512 + 512]

            kr = k_sb[:].rearrange("p (e c) -> p e c", e=NE)
            qr = q_sb[:].rearrange("p (e c) -> p e c", e=NE)
            for kb in range(ext):
                sp = ps_tile(f"{prefix}sp", "mm", 3)
                for i in range(NE // 2):
                    nc.tensor.matmul(
                        sp[:],
                        kr[:, 2 * i:2 * i + 2, kb * P:kb * P + P],
                        qr[:, 2 * i:2 * i + 2, t * 512:t * 512 + 512],
                        start=(i == 0), stop=(i == NE // 2 - 1), perf_mode=DR)
                if masked and kb >= 8 * t:
                    mo = (kb - 8 * t) * 512
                    mt = st5.tile([P, 512], BF16, name=f"{prefix}mt{t}_{kb}",
                                  tag="s5")
                    nc.sync.dma_start(mt[:], d["mask"][t, :, mo:mo + 512])
                    nc.vector.tensor_add(sp[:], sp[:], mt[:])
                nc.scalar.activation(et_sl(kb), sp[:], ACT_F.Exp)
            # softmax denominator: ones-matmul column sums (pre-broadcast);
            # 1/sum is folded into the AV PSUM evacuation below
            sm = ps_tile(f"{prefix}sm", "sx", 2)
            for kb in range(ext):
                nc.tensor.matmul(sm[:], ones_1[:], et_sl(kb),
                                 start=(kb == 0), stop=(kb == ext - 1))
            inv = invp.tile([P, 512], F32, name=f"{prefix}inv{t}",
                            tag=f"i{t}")
            nc.vector.reciprocal_approx_fast(inv[:], sm[:])
            vr = v_sb[:].rearrange("p (k c) -> p k c", k=NKB)
            for af in range(NE):
                ap_ = ps_tile(f"{prefix}avp", "av", 2)
                for jj in range(ext // 2):
                    etr = ets[(2 * jj) // 8][:].rearrange(
                        "p (k c) -> p k c", k=8)
                    kk = (2 * jj) % 8
                    nc.tensor.matmul(
                        ap_[:],
                        vr[:, 2 * jj:2 * jj + 2, af * P:af * P + P],
                        etr[:, kk:kk + 2, :],
                        start=(jj == 0), stop=(jj == ext // 2 - 1),
                        perf_mode=DR)
                o = af * RO + t * 512
                nc.vector.tensor_mul(attn[:, o:o + 512], ap_[:], inv[:])

    def o_proj_residual(attn, wo, res_getter, tag, after_rc=None):
        """h[of,rc] (bf16) = W_o.T @ attn + residual, rc-major."""
        for rc in range(2):
            for of in range(NE):
                op = ps_tile(f"{tag}op", "mm", 3)
                for ab in range(NE):
                    nc.tensor.matmul(
                        op[:],
                        wo[:, ab * E + of * P:ab * E + of * P + P],
                        attn[:, ab * RO + rc * 512:ab * RO + rc * 512 + 512],
                        start=(ab == 0), stop=(ab == NE - 1))
                o = of * RO + rc * 512
                nc.vector.tensor_add(h[:, o:o + 512], op[:],
                                     res_getter(of, rc))
            if after_rc is not None:
                after_rc(rc)

    def ln_sq(rc, prefix):
        """GpSimd squares of one owned 512-chunk of h (for LN sum(x^2))."""
        sqs = []
        for eb in range(NE):
            sl = h[:, eb * RO + rc * 512:eb * RO + rc * 512 + 512]
            sq = sq8.tile([P, 512], BF16, name=f"{prefix}sq{eb}", tag="sq")
            nc.gpsimd.tensor_mul(sq[:], sl, sl)
            sqs.append(sq)
        return sqs

    # ============== self-attention + O1 ====================================
    attn1 = qxp.tile([P, NE * RO], BF16, name="attn1", tag="qx")
    # prefetch: slots for these free as phase-A weights die
    wo1 = w_tile("wo1", BF16)
    wq2 = w_tile("wq2")
    wk2 = w_tile("wk2")

    attention(q1, k1, v1, True, "sa", attn1)

    sq2 = [None, None]

    def after_o1(rc):
        sq2[rc] = ln_sq(rc, f"l2p{rc}")

    def res1(of, rc):
        rt = st5.tile([P, 512], BF16, name=f"res{of}_{rc}", tag="s5")
        o = of * RO + rc * 512
        nc.sync.dma_start(rt[:], d["tgto"][:, o:o + 512])
        return rt[:]

    o_proj_residual(attn1, wo1, res1, "o1", after_rc=after_o1)

    # prefetch (slots free at O1 end / Q2 end)
    wv2 = w_tile("wv2")
    wo2 = w_tile("wo2", BF16)

    # ============== LN2 + K2/Q2/V2 =========================================
    xn2 = qxp.tile([P, NE * RO], F8, name="xn2", tag="qx")
    q2 = qxp.tile([P, NE * RO], F8, name="q2", tag="qx")
    k2 = kvp.tile([P, NE * S], F8, name="k2", tag="kv")
    v2 = kvp.tile([P, NKB * E], F8, name="v2", tag="kv")

    def load_src(rc, nm):
        tiles = xep.tile([P, NE * 512], F8, name=nm, tag="xe")
        for eb in range(NE):
            nc.sync.dma_start(tiles[:, eb * 512:eb * 512 + 512],
                              d["src_t"][rc, eb])
        return tiles

    a2b2 = [stat_tiles(f"l2{rc}") for rc in range(2)]

    def stats2(rc):
        a, b = a2b2[rc]
        ln_stats(lambda eb: h[:, eb * RO + rc * 512:eb * RO + rc * 512 + 512],
                 lambda eb: sq2[rc][eb][:], a, b, f"l2s{rc}")

    def k2_proj(rc, src_rc):
        for kf in range(NE):
            kp = ps_tile("kp2", "mm", 3)
            for i in range(NE // 2):
                nc.tensor.matmul(
                    kp[:], wpair(wk2, i, kf * P, P), xnpair(src_rc, i, 0, 512),
                    start=(i == 0), stop=(i == NE // 2 - 1), perf_mode=DR)
            nc.scalar.activation(
                k2[:, kf * S + rc * 512:kf * S + rc * 512 + 512], kp[:],
                ACT_F.Copy, scale=IS)

    srcK = load_src(0, "srcK0")
    stats2(0)
    k2_proj(0, srcK)
    srcK1 = load_src(1, "srcK1")
    stats2(1)
    for rc in range(2):
        a, b = a2b2[rc]
        for eb in range(NE):
            o = eb * RO + rc * 512
            ln_apply(xn2[:, o:o + 512], h[:, o:o + 512], a, b, f"l2a{rc}")
    k2_proj(1, srcK1)
    srcK2 = load_src(2, "srcK2")
    k2_proj(2, srcK2)
    srcK3 = load_src(3, "srcK3")
    k2_proj(3, srcK3)
    # Q2 projection (owned rows)
    xn2r = xn2[:].rearrange("p (e c) -> p e c", e=NE)
    for fblk in range(NE):
        for rc in range(2):
            qp = ps_tile("q2p", "mm", 3)
            for i in range(NE // 2):
                nc.tensor.matmul(
                    qp[:], wpair(wq2, i, fblk * P, P),
                    xn2r[:, 2 * i:2 * i + 2, rc * 512:rc * 512 + 512],
                    start=(i == 0), stop=(i == NE // 2 - 1), perf_mode=DR)
            o = fblk * RO + rc * 512
            nc.scalar.activation(q2[:, o:o + 512], qp[:], ACT_F.Copy,
                                 scale=IS)
    # V2 (re-stream src chunks)
    for rc in range(4):
        src_rc = load_src(rc, f"srcV{rc}")
        for rb in range(4):
            for vf in range(2):
                vp = ps_tile("vp2", "mm", 3)
                for i in range(NE // 2):
                    nc.tensor.matmul(
                        vp[:], xnpair(src_rc, i, rb * P, P),
                        wpair(wv2, i, vf * 512, 512),
                        start=(i == 0), stop=(i == NE // 2 - 1), perf_mode=DR)
                o = (rc * 4 + rb) * E + vf * 512
                nc.scalar.activation(v2[:, o:o + 512], vp[:], ACT_F.Copy,
                                     scale=IS)

    # ============== cross-attention + O2 (in-place residual) ===============
    attn2 = qxp.tile([P, NE * RO], BF16, name="attn2", tag="qx")

    # w1 stream prefetch (fresh slots, DMAs run during CA)
    w1_tiles = {}

    def w1_tile(fb):
        if fb not in w1_tiles:
            t = w1p.tile([P, NE * P], BF16, name=f"w1t{fb}", tag="w1")
            nc.sync.dma_start(t[:], d["w1"][fb])
            w1_tiles[fb] = t
        return w1_tiles[fb]

    for fb in range(3):
        w1_tile(fb)

    attention(q2, k2, v2, False, "ca", attn2)

    sq3 = [None, None]

    def after_o2(rc):
        sq3[rc] = ln_sq(rc, f"l3p{rc}")

    o_proj_residual(attn2, wo2,
                    lambda of, rc: h[:, of * RO + rc * 512:
                                     of * RO + rc * 512 + 512],
                    "o2", after_rc=after_o2)

    # ============== LN3 + FFN + final residual =============================
    xn3 = qxp.tile([P, NE * RO], BF16, name="xn3", tag="qx")
    hft_a = kvp.tile([P, 16 * RO], BF16, name="hft_a", tag="kv")
    hft_b = kvp.tile([P, 16 * RO], BF16, name="hft_b", tag="kv")

    def hft_sl(fb, rc):
        t = hft_a if fb < 16 else hft_b
        o = (fb % 16) * RO + rc * 512
        return t[:, o:o + 512]

    def hft_pair(j, rc):
        """[P,2,512] fb-block pair (2j,2j+1) of hft."""
        t = hft_a if 2 * j < 16 else hft_b
        r = t[:].rearrange("p (f c) -> p f c", f=16)
        jj = (2 * j) % 16
        return r[:, jj:jj + 2, rc * 512:rc * 512 + 512]

    a3b3 = [stat_tiles(f"l3{rc}") for rc in range(2)]

    def apply3(rc):
        a, b = a3b3[rc]
        for eb in range(NE):
            o = eb * RO + rc * 512
            ln_apply(xn3[:, o:o + 512], h[:, o:o + 512], a, b, f"l3a{rc}")

    for rc in range(2):
        a, b = a3b3[rc]
        ln_stats(lambda eb: h[:, eb * RO + rc * 512:eb * RO + rc * 512 + 512],
                 lambda eb: sq3[rc][eb][:], a, b, f"l3s{rc}")
        apply3(rc)

    # FF1: first rc1-groups deferred so apply3(rc1) hides behind rc0 work
    ff1_order = [(0, 0), (1, 0), (0, 1), (1, 1)] + \
        [(fb, rc) for fb in range(2, NF) for rc in range(2)]
    xn3r = xn3[:].rearrange("p (e c) -> p e c", e=NE)
    for fb, rc in ff1_order:
        w1t = w1_tile(fb)
        if rc == 0 and fb + 2 < NF:
            w1_tile(fb + 2)  # keep the w1 DMA stream two tiles ahead
        hps = ps_tile("hps", "mm", 3)
        for eb in range(NE):
            nc.tensor.matmul(
                hps[:],
                w1t[:, eb * P:eb * P + P],
                xn3[:, eb * RO + rc * 512:eb * RO + rc * 512 + 512],
                start=(eb == 0), stop=(eb == NE - 1))
        nc.scalar.activation(hft_sl(fb, rc), hps[:], ACT_F.Relu)

    # FF2 + final residual in fp32 + chunked output DMA
    w2_tiles = []

    def w2_prefetch(upto):
        while len(w2_tiles) < min(upto, NE):
            j = len(w2_tiles)
            t = warena.tile([P, NF * P], BF16, name=f"w2t{j}", tag="w")
            nc.sync.dma_start(t[:], d["w2"][j])
            w2_tiles.append(t)

    w2_prefetch(2)
    for of in range(NE):
        w2_prefetch(of + 3)
        w2t = w2_tiles[of]
        for rc in range(2):
            ops = ps_tile("ops", "mm", 3)
            for fb in range(NF):
                nc.tensor.matmul(
                    ops[:],
                    w2t[:, fb * P:fb * P + P],
                    hft_sl(fb, rc),
                    start=(fb == 0), stop=(fb == NF - 1))
            o = of * RO + rc * 512
            ot = outp.tile([P, 512], F32, name=f"out{of}_{rc}", tag="ot")
            nc.vector.tensor_add(ot[:], ops[:], h[:, o:o + 512])
            nc.sync.dma_start(d["out_t"][:, o:o + 512], ot[:])

    for p_ in (w1p, hpool, xep, qxp, kvp, warena, outp, st5, invp, statp,
               sq8, tmp, consts, ps):
        p_.release()


# ---------------------------------------------------------------------------
# host side: input swizzling, weight folding, output assembly
# ---------------------------------------------------------------------------

def _swz_w(w_t):
    """[E_in, N] (already [in, out]) -> SBUF image [P, (E_in/P)*N]."""
    e_in, n = w_t.shape
    return np.ascontiguousarray(
        w_t.reshape(e_in // P, P, n).transpose(1, 0, 2).reshape(P, -1))


def _own_rows(h):
    idx = []
    for j in range(8):
        g = 2 * j + h
        idx.extend(range(g * P, (g + 1) * P))
    return np.array(idx)


# swap even/odd 128-row groups: [1,0,3,2,5,4,...]
_BLKSWAP = np.arange(NKB).reshape(-1, 2)[:, ::-1].reshape(-1)


def _chunked(x_t):
    """[E, S] -> [4, NE, P, 512] (rc-chunk major, feature-block, part)."""
    return np.ascontiguousarray(
        x_t.reshape(NE, P, 4, 512).transpose(2, 0, 1, 3))


def make_in_maps(inputs):
    f32 = np.float32
    tgt = np.asarray(inputs["tgt_embs"], f32)
    src = np.asarray(inputs["src_encs"], f32)

    g1 = np.asarray(inputs["ln1_g"], f32)
    g2 = np.asarray(inputs["ln2_g"], f32)
    g3 = np.asarray(inputs["ln3_g"], f32)
    for nm in ("sa_bq", "sa_bk", "sa_bv", "sa_bo", "ca_bq", "ca_bk", "ca_bv",
               "ca_bo", "ff_b1", "ff_b2", "ln1_b", "ln2_b", "ln3_b"):
        assert np.abs(np.asarray(inputs[nm])).max() == 0.0, \
            f"nonzero bias {nm} not supported"

    scale = f32(1.0 / np.sqrt(E))
    wq1 = np.asarray(inputs["sa_Wq"], f32) * g1[None, :] * scale
    wk1 = np.asarray(inputs["sa_Wk"], f32) * g1[None, :]
    wv1 = np.asarray(inputs["sa_Wv"], f32) * g1[None, :]
    wo1 = np.asarray(inputs["sa_Wo"], f32)
    wq2 = np.asarray(inputs["ca_Wq"], f32) * g2[None, :] * scale
    wk2 = np.asarray(inputs["ca_Wk"], f32)
    wv2 = np.asarray(inputs["ca_Wv"], f32)
    wo2 = np.asarray(inputs["ca_Wo"], f32)
    w1 = np.asarray(inputs["ff_W1"], f32) * g3[None, :]
    w2 = np.asarray(inputs["ff_W2"], f32)

    ws = np.float32(WS)
    w_sb = {
        "wq1": _swz_w((wq1.T * ws).astype(F8NP)),
        "wk1": _swz_w((wk1.T * ws).astype(F8NP)),
        "wv1": _swz_w((wv1.T * ws).astype(F8NP)),
        "wo1": _swz_w(wo1.T.astype(BF)),
        "wq2": _swz_w((wq2.T * ws).astype(F8NP)),
        "wk2": _swz_w((wk2.T * ws).astype(F8NP)),
        "wv2": _swz_w((wv2.T * ws).astype(F8NP)),
        "wo2": _swz_w(wo2.T.astype(BF)),
    }
    w1t = w1.T.astype(BF)  # [E, F]
    w1_sw = np.ascontiguousarray(
        w1t.reshape(NE, P, NF, P).transpose(2, 1, 0, 3).reshape(NF, P, NE * P))
    w2t = w2.T.astype(BF)  # [F, E]
    w2_sw = np.ascontiguousarray(
        w2t.reshape(NF, P, NE, P).transpose(2, 1, 0, 3).reshape(NE, P, NF * P))

    in_maps = []
    for c in range(NCORES):
        b, h = c // 2, c % 2
        rows = _own_rows(h)
        # perm: physical row position -> original row index (h=1 swaps each
        # even/odd 128-row group pair so owned groups land at even positions)
        if h == 1:
            perm = (_BLKSWAP[:, None] * P + np.arange(P)[None, :]).reshape(-1)
        else:
            perm = np.arange(S)
        tgt_t = _chunked(tgt[b][perm].T).astype(BF)
        tgto = _swz_w(np.ascontiguousarray(tgt[b][rows].T)).astype(BF)
        src_t = _chunked(src[b].T).astype(F8NP)
        mask = np.zeros((2, 8, P, 512), np.float32)
        for t in range(2):
            kr = perm[1024 * t:1024 * t + 1024]  # original index of each key
            qg = np.empty(512, np.int64)
            for s in range(4):
                g = 8 * t + 2 * s + h
                qg[s * P:(s + 1) * P] = g * P + np.arange(P)
            m = np.where(kr[:, None] <= qg[None, :], 0.0, NEG).astype(np.float32)
            mask[t] = m.reshape(8, P, 512)
        # kernel mask layout: [t, P, kb*512]
        mask_k = np.ascontiguousarray(mask.transpose(0, 2, 1, 3)
                                      .reshape(2, P, 8 * 512))
        in_maps.append({
            "tgt_t": tgt_t,
            "tgto": tgto,
            "src_t": src_t,
            "mask": mask_k.astype(BF),
            **w_sb,
            "w1": w1_sw,
            "w2": w2_sw,
        })
    return in_maps


def assemble_output(results):
    out = np.empty((B, S, E), np.float32)
    for c in range(NCORES):
        b, h = c // 2, c % 2
        arr = np.asarray(results[c]["out_t"])  # [P, NE*RO]
        a = arr.reshape(P, NE, 8, P).transpose(2, 3, 1, 0).reshape(8, P, E)
        for j in range(8):
            g = 2 * j + h
            out[b, g * P:(g + 1) * P, :] = a[j]
    return out


def get_nc():
    if "nc" not in _NC_CACHE:
        _NC_CACHE["nc"] = _build_program()
    return _NC_CACHE["nc"]


def _axon_reset():
    """Recover a wedged remote NeuronCore (NRT_EXEC_UNIT_UNRECOVERABLE)."""
    try:
        import ctypes
        lib = ctypes.CDLL("/opt/axon/libaxon_pjrt.so")
        lib.axon_reset.restype = ctypes.c_int64
        lib.axon_reset()
    except Exception:
        pass


def kernel(**inputs):
    global LAST_RESULTS
    in_maps = make_in_maps(inputs)
    nc = get_nc()
    last_err = None
    for attempt in range(3):
        try:
            res = run_bass_kernel_spmd(nc, in_maps, list(range(NCORES)))
            break
        except Exception as e:  # wedged device -> reset + retry
            last_err = e
            _axon_reset()
    else:
        raise last_err
    LAST_RESULTS = res
    return assemble_output(res.results)
